# revision 22
# baseline (speedup 1.0000x reference)
"""Trainium2 Bass kernel for a dense transformer block (nn_Block_3453153706485).

B=4, S=1024, D=1024, H=16 heads (hd=64), FF=4096, fp32 I/O.
Sharding: 8 cores; core c owns (batch b=c//2, token half c%2) -> 512 query
tokens.

Key optimization: ~50% of keys are masked out by the key-padding mask and
contribute nothing.  The HOST gathers the unmasked key tokens per batch
(padded to a multiple of 128 with duplicates of token 0 carrying keep=0) so
k/v projections, scores, exp and p@v run only on ~5/8 of the sequence.

All matmuls bf16 (residual fp32).  Transposes run on the DMA XBAR
(dma_start_transpose, sync queue).  LayerNorm rstd = exp(-0.5*ln(var+eps))
so the whole pre-MLP kernel stays on one ACT table set (natural_log_exp).
Softmax: p@v head pairs accumulate into one 2-bank psum with a keep-mask
row giving the denominator Z; Z rows are batch-reciprocal'd, staged through
DRAM, broadcast, and attnT is normalized in place in bf16.
"""

from contextlib import ExitStack

import ml_dtypes
import numpy as np

import concourse.bass as bass
import concourse.tile as tile
from concourse import bacc, mybir

# Make Exp and Ln resolve to the single combined ACT table set so the
# whole pre-MLP kernel needs one table load (the greedy per-function set
# picker would otherwise thrash natural_log <-> exp_and_others per LN).
import concourse.hw_specs as _hw_specs
import concourse.bacc as _bacc_mod
import concourse.bass_interp as _interp_mod

_orig_gat = _hw_specs.get_activation_tables


def _gat_combined(arch):
    _AF = mybir.ActivationFunctionType
    tables = {}
    for name, fns in _orig_gat(arch).items():
        fns = set(fns)
        if name != "natural_log_exp_and_others":
            fns.discard(_AF.Exp)
            fns.discard(_AF.Ln)
        tables[name] = fns
    return tables


_bacc_mod.get_activation_tables = _gat_combined
_interp_mod.get_activation_tables = _gat_combined

F32 = mybir.dt.float32
BF16 = mybir.dt.bfloat16
AF = mybir.ActivationFunctionType
OP = mybir.AluOpType

P = 128
D = 1024
H = 16
HD = 64
FF = 4096
TL = 512           # local (query) tokens per core
KC = D // P        # 8
NCORES = 8
EPS = 1e-5


def _swap_pairs(ap4):
    """View with the two elements of each innermost [step,2] pair swapped."""
    st = ap4.ap[-1][0]
    return bass.AP(
        tensor=ap4.tensor,
        offset=ap4.offset + st,
        ap=list(ap4.ap[:-1]) + [[-st, 2]],
    )


def build_program(nkc: int, apply_ln1: bool, apply_ln2: bool,
                  sim_compat: bool = False):
    nc = bacc.Bacc("TRN2", target_bir_lowering=False, debug=False)
    nk = nkc * P
    NQ = TL // P       # 4

    xg = nc.dram_tensor("xg", [nk, D], BF16, kind="ExternalInput").ap()
    xq = nc.dram_tensor("xq", [TL, D], F32, kind="ExternalInput").ap()
    mmz_d = nc.dram_tensor("mmz", [P, nkc, 2], F32, kind="ExternalInput").ap()
    trig_d = nc.dram_tensor("trig", [P, 2 * nkc + 8, HD], BF16,
                            kind="ExternalInput").ap()
    wq_d = nc.dram_tensor("wq", [P, KC, D], BF16, kind="ExternalInput").ap()
    wk_d = nc.dram_tensor("wk", [P, KC, D], BF16, kind="ExternalInput").ap()
    wv_d = nc.dram_tensor("wv", [P, KC, D], BF16, kind="ExternalInput").ap()
    wo_d = nc.dram_tensor("wo", [P, KC, D], BF16, kind="ExternalInput").ap()
    w1_d = nc.dram_tensor("w1", [P, KC, FF], BF16, kind="ExternalInput").ap()
    w2_d = nc.dram_tensor("w2", [P, FF // P, D], BF16, kind="ExternalInput").ap()
    ln_d = {nm: nc.dram_tensor(nm, [1, D], F32, kind="ExternalInput").ap()
            for nm in ("ln1w", "ln1b", "ln2w", "ln2b")}
    out_d = nc.dram_tensor("out", [TL, D], F32, kind="ExternalOutput").ap()
    zs_d = nc.dram_tensor("zstage", [H, TL], BF16, kind="Internal").ap()

    gelu_f = AF.Identity if sim_compat else AF.Gelu

    with tile.TileContext(nc) as tc:
        es0 = ExitStack()

        cons = es0.enter_context(tc.tile_pool(name="cons", bufs=1))
        work = es0.enter_context(tc.tile_pool(name="work", bufs=2))

        # ---- x loads first (they gate everything) ----
        pool_hT = tc.alloc_tile_pool(name="p_hT", bufs=1, side="right")
        pool_xg = tc.alloc_tile_pool(name="p_xg", bufs=1, side="right")
        xts = []
        for ti in range(nkc):
            xt = pool_xg.tile([P, D], BF16, name=f"xt{ti}")
            nc.sync.dma_start(xt[:], xg[ti * P:(ti + 1) * P, :])
            xts.append(xt)
        # ---------------- constants ----------------
        eps_t = cons.tile([P, 1], F32, name="eps_t")
        nc.vector.memset(eps_t[:], EPS)

        pool_kT = tc.alloc_tile_pool(name="p_kT", bufs=1)
        pool_v1 = tc.alloc_tile_pool(name="p_v1", bufs=1)
        pool_qT = tc.alloc_tile_pool(name="p_qT", bufs=1)

        # prefetch all q/k/v weight halves (bf16: 6 x 8KB/partition); then
        # xq + trig constants, all on the gpsimd queue so the ACT queue has
        # no DMA issues in front of the LayerNorm chain
        pool_wB = tc.alloc_tile_pool(name="p_wB", bufs=1)
        wkh, wvh, wqh = [], [], []
        for nm, lst, src in (("wk", wkh, wk_d), ("wv", wvh, wv_d),
                             ("wq", wqh, wq_d)):
            for nh in range(2):
                t = pool_wB.tile([P, KC, 512], BF16, name=f"{nm}{nh}")
                nc.gpsimd.dma_start(t[:], src[:, :, nh * 512:(nh + 1) * 512])
                lst.append(t)

        xqs = []
        for ti in range(NQ):
            xtq = work.tile([P, D], F32, tag="xtq", bufs=NQ, name=f"xtq{ti}")
            nc.gpsimd.dma_start(xtq[:], xq[ti * P:(ti + 1) * P, :])
            xqs.append(xtq)

        trig = cons.tile([P, 2 * nkc + 8, HD], BF16, name="trig_sb")
        nc.gpsimd.dma_start(trig[:], trig_d)
        cik = trig[:, 0:nkc, :]
        sik = trig[:, nkc:2 * nkc, :]
        ciq = trig[:, 2 * nkc:2 * nkc + 4, :]
        siq = trig[:, 2 * nkc + 4:2 * nkc + 8, :]
        mmz = cons.tile([P, nkc, 2], F32, name="mmz_sb")
        nc.gpsimd.dma_start(mmz[:], mmz_d)

        ln_bc = {}
        for nm, need in (("ln1w", apply_ln1), ("ln1b", apply_ln1),
                         ("ln2w", apply_ln2), ("ln2b", apply_ln2)):
            if need:
                t = cons.tile([P, D], F32, name=f"{nm}_bc")
                src = bass.AP(tensor=ln_d[nm].tensor, offset=ln_d[nm].offset,
                              ap=[[0, P], [1, D]])
                nc.gpsimd.dma_start(t[:], src)
                ln_bc[nm] = t

        def layernorm(src_ap, dst_ap, wkey, bkey, applied):
            """src [P, D] (any dtype) -> dst [P, D] bf16 normalized.

            rstd = exp(-0.5*ln(var+eps)): stays on the natural_log_exp ACT
            table set, so no table switches between LN / softmax-exp.
            """
            stats = work.tile([P, 2, 6], F32, tag="stats", bufs=4, name="st")
            nc.vector.bn_stats(stats[:, 0, :], src_ap[:, 0:512])
            nc.vector.bn_stats(stats[:, 1, :], src_ap[:, 512:1024])
            mv = work.tile([P, 2], F32, tag="mv", bufs=4, name="mv")
            nc.vector.bn_aggr(mv[:], stats[:])
            std = work.tile([P, 1], F32, tag="std", bufs=4, name="std")
            nc.scalar.activation(std[:], mv[:, 1:2], AF.Sqrt, bias=eps_t[:])
            rstd = work.tile([P, 1], F32, tag="rstd", bufs=4, name="rstd")
            nc.vector.reciprocal(rstd[:], std[:])
            nc.vector.tensor_scalar(dst_ap, src_ap, mv[:, 0:1], rstd[:],
                                    OP.subtract, OP.mult)
            if applied:
                nc.vector.tensor_tensor(dst_ap, dst_ap, ln_bc[wkey][:], OP.mult)
                nc.vector.tensor_tensor(dst_ap, dst_ap, ln_bc[bkey][:], OP.add)

        def rope(krb, ci, si, ti, dst_ap, nh8):
            """RoPE a [P, nh8*64] bf16 tile -> dst bf16 sbuf."""
            w = nh8 * HD
            kb_h = krb.rearrange("p (h i) -> p h i", h=nh8)
            ci_b = ci[:, ti, None, :].to_broadcast((P, nh8, HD))
            p1 = work.tile([P, w], BF16, tag=f"p1_{nh8}", bufs=2, name="p1")
            nc.vector.tensor_tensor(p1.rearrange("p (h i) -> p h i", h=nh8),
                                    kb_h, ci_b, OP.mult)
            kb_sw = _swap_pairs(krb.rearrange("p (h i two) -> p h i two",
                                              h=nh8, two=2))
            si_b = (si[:, ti, None, :].to_broadcast((P, nh8, HD))
                    .rearrange("p h (i two) -> p h i two", two=2))
            p2 = work.tile([P, w], BF16, tag=f"p2_{nh8}", bufs=2, name="p2")
            nc.vector.tensor_tensor(
                p2.rearrange("p (h i two) -> p h i two", h=nh8, two=2),
                kb_sw, si_b, OP.mult)
            nc.vector.tensor_tensor(dst_ap, p1[:], p2[:], OP.add)

        def proj_group(dst_ps, lhs_base, wtile):
            for kc in range(KC):
                nc.tensor.matmul(dst_ps, lhsT=lhs_base[:, kc, :],
                                 rhs=wtile[:, kc, :],
                                 start=(kc == 0), stop=(kc == KC - 1))

        psB = tc.alloc_tile_pool(name="psB", bufs=1, space="PSUM")

        def ps512(nm):
            return psB.tile([P, 512], F32, tag="ps512", bufs=4, name=nm)

        # ========== stage A: LN1 of gathered key tokens -> hT ==========
        hT = pool_hT.tile([P, nkc, KC, P], BF16, name="hT")
        for ti in range(nkc):
            h = work.tile([P, D], BF16, tag="h", bufs=3, name=f"h{ti}")
            layernorm(xts[ti][:], h[:], "ln1w", "ln1b", apply_ln1)
            nc.sync.dma_start_transpose(hT[:, ti, :, :], h[:])
        pool_xg.release()

        # ========== stage B: k, v (gathered keys), q (local) ==========
        kT = pool_kT.tile([P, nkc, KC, P], BF16, name="kT")
        for ti in range(nkc):
            krb = work.tile([P, D], BF16, tag="krb", bufs=2, name=f"krb{ti}")
            for nh in range(2):
                ps = ps512(f"kps{nh}_{ti}")
                proj_group(ps, hT[:, ti, :, :], wkh[nh])
                nc.scalar.copy(krb[:, nh * 512:(nh + 1) * 512], ps[:])
            kr = work.tile([P, D], BF16, tag="kr", bufs=2, name=f"kr{ti}")
            rope(krb[:], cik, sik, ti, kr[:], 16)
            nc.sync.dma_start_transpose(kT[:, ti, :, :], kr[:])

        v1 = pool_v1.tile([P, nkc, H, 66], BF16, name="v1")
        # v1 mask columns: col 64 = keep flag (1/0), col 65 = 0 (pad)
        for ti in range(nkc):
            nc.vector.tensor_copy(v1[:, ti, :, 64:66],
                                  mmz[:, ti, None, :].to_broadcast((P, H, 2)))
        for nh in range(2):
            for ti in range(nkc):
                ps = ps512(f"vps{nh}_{ti}")
                proj_group(ps, hT[:, ti, :, :], wvh[nh])
                # eviction with per-token keep-scale on the ACT engine
                nc.scalar.activation(
                    v1[:, ti, nh * 8:(nh + 1) * 8, 0:64],
                    ps.rearrange("p (h d) -> p h d", h=8),
                    AF.Copy, scale=mmz[:, ti, 0:1])

        # q path: nh-outer so scores for head-pairs 0-3 can start while the
        # nh=1 projections still run
        qT = pool_qT.tile([P, NQ, KC, P], BF16, name="qT")
        hqTs = []
        for ti in range(NQ):
            hq = work.tile([P, D], BF16, tag="h", bufs=3, name=f"hq{ti}")
            layernorm(xqs[ti][:], hq[:], "ln1w", "ln1b", apply_ln1)
            hqT = work.tile([P, KC, P], BF16, tag="hqT", bufs=NQ,
                            name=f"hqT{ti}")
            nc.sync.dma_start_transpose(hqT[:], hq[:])
            hqTs.append(hqT)
        for nh in range(2):
            for ti in range(NQ):
                ps = ps512(f"qps{nh}_{ti}")
                proj_group(ps, hqTs[ti], wqh[nh])
                qrb = work.tile([P, 512], BF16, tag="qrb", bufs=2,
                                name=f"qrb{nh}_{ti}")
                nc.scalar.copy(qrb[:], ps[:])
                qr = work.tile([P, 512], BF16, tag="qr", bufs=3,
                               name=f"qr{nh}_{ti}")
                rope(qrb[:], ciq, siq, ti, qr[:], 8)
                nc.sync.dma_start_transpose(
                    qT[:, ti, nh * 4:(nh + 1) * 4, :], qr[:])

        pool_wB.release()
        pool_hT.release()
        psB.release()

        # ========== stage C: attention per head-pair ==========
        pool_wD = tc.alloc_tile_pool(name="p_wD", bufs=1, side="right")
        woh = []
        for nh in range(2):
            t = pool_wD.tile([P, KC, 512], BF16, name=f"wo{nh}")
            nc.gpsimd.dma_start(t[:], wo_d[:, :, nh * 512:(nh + 1) * 512])
            woh.append(t)

        psC = tc.alloc_tile_pool(name="psC", bufs=1, space="PSUM")
        pool_at = tc.alloc_tile_pool(name="p_at", bufs=1, side="right")
        pool_pT = tc.alloc_tile_pool(name="p_pT", bufs=2, side="right")
        pool_z = tc.alloc_tile_pool(name="p_z", bufs=1, side="right")
        pool_zbc = tc.alloc_tile_pool(name="p_zbc", bufs=1, side="right")
        attnT = pool_at.tile([P, KC, TL], BF16, name="attnT")
        ztabs = {}

        def extract_z(pj, pvv):
            # Z rows -> ztab rows {0,32,64,96} (DVE)
            for eo in range(2):
                hh = 2 * pj + eo
                hg, hi = divmod(hh, 4)
                if hi == 0:
                    ztabs[hg] = pool_z.tile([P, TL], F32, tag="ztab", bufs=2,
                                            name=f"ztab{hg}")
                    nc.vector.memset(ztabs[hg][:], 1.0)
                nc.vector.tensor_copy(ztabs[hg][32 * hi:32 * hi + 1, :],
                                      pvv[64:65, eo * 512:(eo + 1) * 512])

        def recip_stage(hg):
            # batched reciprocal for 4 heads (rows 0/32/64/96); cast to
            # bf16; stage through DRAM
            ztab = ztabs.pop(hg)
            nc.vector.reciprocal(ztab[:], ztab[:])
            zcast = pool_z.tile([P, TL], BF16, tag="zcast", bufs=2,
                                name=f"zcast{hg}")
            nc.vector.tensor_copy(zcast[:], ztab[:])
            for hi in range(4):
                hh = hg * 4 + hi
                dst = bass.AP(tensor=zs_d.tensor,
                              offset=zs_d.offset + hh * TL,
                              ap=[[TL, 1], [1, TL]])
                nc.sync.dma_start(dst, zcast[32 * hi:32 * hi + 1, :])

        def evict_attn(pj, pvv):
            # unnormalized attnT evict (DVE, bf16 cast)
            for eo in range(2):
                pb = 64 * eo
                nc.vector.tensor_copy(attnT[pb:pb + 64, pj, :],
                                      pvv[0:64, eo * 512:(eo + 1) * 512])

        def emit_normalize(hg):
            # broadcast 1/Z for 4 heads in one DMA; in-place bf16 normalize
            zbc = pool_zbc.tile([P, 4, TL], BF16, tag="zbc", bufs=2,
                                name=f"zbc{hg}")
            src = bass.AP(tensor=zs_d.tensor, offset=zs_d.offset + hg * 4 * TL,
                          ap=[[0, P], [TL, 4], [1, TL]])
            nc.sync.dma_start(zbc[:], src)
            for hi in range(4):
                hh = hg * 4 + hi
                pj, eo = divmod(hh, 2)
                pb = 64 * eo
                nc.vector.tensor_tensor(attnT[pb:pb + 64, pj, :],
                                        attnT[pb:pb + 64, pj, :],
                                        zbc[pb:pb + 64, hi, :], OP.mult)

        prev = None
        for j in range(H // 2 + 1):
            last = j == H // 2
            if not last:
                pt = pool_pT.tile([P, nkc, 2, 512], BF16, tag="pT",
                                  name=f"pT{j}")
            for skc in range(nkc):
                if not last:
                    pss = psC.tile([P, 1024], F32, tag="sc", bufs=2,
                                   name=f"scps{j}_{skc}")
                    for eo in range(2):
                        pb = 64 * eo
                        nc.tensor.matmul(
                            pss[:, eo * 512:(eo + 1) * 512],
                            lhsT=kT[pb:pb + 64, skc, j, :],
                            rhs=qT[pb:pb + 64, :, j, :],
                            start=True, stop=True)
                    nc.scalar.activation(
                        pt[:, skc, :, :].rearrange("p a b -> p (a b)"),
                        pss[:], AF.Exp, scale=0.125)
                if prev is not None:
                    pj, ppt, pvv = prev
                    for eo in range(2):
                        nc.tensor.matmul(
                            pvv[0:66, eo * 512:(eo + 1) * 512],
                            lhsT=v1[:, skc, 2 * pj + eo, :],
                            rhs=ppt[:, skc, eo, :],
                            start=(skc == 0), stop=(skc == nkc - 1))
            if prev is not None:
                pj = prev[0]
                extract_z(pj, prev[2])
                if pj % 2 == 1:
                    recip_stage(pj // 2)
                evict_attn(pj, prev[2])
                if pj % 2 == 1:
                    emit_normalize(pj // 2)
            if not last:
                prev = (j, pt, psC.tile([P, 1024], F32, tag="pv", bufs=2,
                                        name=f"pv{j}"))

        pool_zbc.release()
        pool_z.release()
        pool_pT.release()
        pool_qT.release()
        pool_v1.release()
        pool_kT.release()
        psC.release()

        # ========== stage D: wo + residual -> xres; LN2 -> h2T ==========
        psD = tc.alloc_tile_pool(name="psD", bufs=1, space="PSUM")
        pool_res = tc.alloc_tile_pool(name="p_res", bufs=1)
        xres = pool_res.tile([P, NQ, D], F32, name="xres")
        h2T = pool_res.tile([P, NQ, KC, P], BF16, name="h2T")
        pool_w1 = tc.alloc_tile_pool(name="p_w1", bufs=2)

        def psDt(nm):
            return psD.tile([P, 512], F32, tag="wops", bufs=2, name=nm)

        # wo split: heads 0-11 (kc 0-5) are normalized well before heads
        # 12-15 (kc 6-7); run the early part as soon as psum frees, then
        # only the 2-chunk remainder waits on the last softmax group
        for tc4 in range(NQ):
            for nh in range(2):
                ps = psDt(f"woA{tc4}_{nh}")
                for kc in range(6):
                    nc.tensor.matmul(
                        ps[:], lhsT=attnT[:, kc, tc4 * P:(tc4 + 1) * P],
                        rhs=woh[nh][:, kc, :],
                        start=(kc == 0), stop=(kc == 5))
                nc.vector.tensor_add(xres[:, tc4, nh * 512:(nh + 1) * 512],
                                     ps[:],
                                     xqs[tc4][:, nh * 512:(nh + 1) * 512])
        for tc4 in range(NQ):
            for nh in range(2):
                ps = psDt(f"woB{tc4}_{nh}")
                for kc in range(6, KC):
                    nc.tensor.matmul(
                        ps[:], lhsT=attnT[:, kc, tc4 * P:(tc4 + 1) * P],
                        rhs=woh[nh][:, kc, :],
                        start=(kc == 6), stop=(kc == KC - 1))
                nc.vector.tensor_add(xres[:, tc4, nh * 512:(nh + 1) * 512],
                                     ps[:],
                                     xres[:, tc4, nh * 512:(nh + 1) * 512])
            h2 = work.tile([P, D], BF16, tag="h", bufs=3, name=f"h2{tc4}")
            layernorm(xres[:, tc4, :], h2[:], "ln2w", "ln2b", apply_ln2)
            nc.sync.dma_start_transpose(h2T[:, tc4, :, :], h2[:])

        pool_at.release()
        pool_wD.release()
        psD.release()
        psE = tc.alloc_tile_pool(name="psE", bufs=1, space="PSUM")

        # ========== stage E: MLP ==========
        pool_g1 = tc.alloc_tile_pool(name="p_g1", bufs=1, side="right")
        pool_w2 = tc.alloc_tile_pool(name="p_w2", bufs=2, side="right")
        g1 = pool_g1.tile([P, FF // P, TL], BF16, name="g1")

        for fg in range(FF // 1024):
            w1c = pool_w1.tile([P, KC, 1024], BF16, tag="w1c", name=f"w1c{fg}")
            nc.scalar.dma_start(w1c[:], w1_d[:, :, fg * 1024:(fg + 1) * 1024])
            for j in range(8):
                ps = psE.tile([P, 512], F32, tag="m1ps", bufs=4,
                              name=f"m1ps{fg}_{j}")
                # split over tc-halves so the first MMs start before the
                # last h2T tile lands
                for c2 in range(2):
                    for kc in range(KC):
                        nc.tensor.matmul(
                            ps[:, c2 * 256:(c2 + 1) * 256],
                            lhsT=w1c[:, kc, j * P:(j + 1) * P],
                            rhs=h2T[:, 2 * c2:2 * c2 + 2, kc, :],
                            start=(kc == 0), stop=(kc == KC - 1))
                nc.scalar.activation(g1[:, fg * 8 + j, :], ps[:], gelu_f)

        ots = []
        for tc4 in range(NQ):
            ots.append(work.tile([P, D], F32, tag="osb", bufs=NQ,
                                 name=f"ot{tc4}"))
        for nh in range(2):
            psos = [psE.tile([P, 512], F32, tag="m2ps", bufs=4,
                             name=f"m2ps{nh}_{tc4}") for tc4 in range(NQ)]
            for kg in range(4):
                w2c = pool_w2.tile([P, 8, 512], BF16, tag="w2c",
                                   name=f"w2c{nh}_{kg}")
                nc.scalar.dma_start(
                    w2c[:], w2_d[:, kg * 8:(kg + 1) * 8,
                                 nh * 512:(nh + 1) * 512])
                for tc4 in range(NQ):
                    for kc in range(8):
                        nc.tensor.matmul(
                            psos[tc4],
                            lhsT=g1[:, kg * 8 + kc, tc4 * P:(tc4 + 1) * P],
                            rhs=w2c[:, kc, :],
                            start=(kg == 0 and kc == 0),
                            stop=(kg == 3 and kc == 7))
            for tc4 in range(NQ):
                nc.vector.tensor_add(
                    ots[tc4][:, nh * 512:(nh + 1) * 512], psos[tc4][:],
                    xres[:, tc4, nh * 512:(nh + 1) * 512])
        for tc4 in range(NQ):
            nc.sync.dma_start(out_d[tc4 * P:(tc4 + 1) * P, :], ots[tc4][:])

        pool_w2.release()
        pool_g1.release()
        pool_w1.release()
        pool_res.release()
        psE.release()
        es0.close()

    nc.compile()
    return nc


# ---------------------------------------------------------------------------
# Host side
# ---------------------------------------------------------------------------

_PROGRAM_CACHE = {}


def _get_program(nkc, apply_ln1, apply_ln2, sim_compat=False):
    key = (nkc, apply_ln1, apply_ln2, sim_compat)
    if key not in _PROGRAM_CACHE:
        _PROGRAM_CACHE[key] = build_program(*key)
    return _PROGRAM_CACHE[key]


def _prep_inputs(x, mask, freqs_cos, freqs_sin, wq, wk, wv, wo, w1, w2,
                 ln1_w, ln1_b, ln2_w, ln2_b):
    """Build the 8 per-core input dicts.  Returns (in_maps, nkc)."""
    f32 = np.float32
    bf16 = ml_dtypes.bfloat16
    x = np.asarray(x, f32)
    mask = np.asarray(mask)
    cos = np.asarray(freqs_cos, f32)
    sin = np.asarray(freqs_sin, f32)
    S = x.shape[1]

    ci = np.empty((S, HD), f32)
    ci[:, 0::2] = cos
    ci[:, 1::2] = cos
    si = np.empty((S, HD), f32)
    si[:, 0::2] = -sin
    si[:, 1::2] = sin

    # gather unmasked key tokens per batch, pad to a common multiple of 128
    idxs, keeps = [], []
    nkc = 1
    for b in range(x.shape[0]):
        idx = np.nonzero(~mask[b])[0]
        nkc = max(nkc, -(-max(len(idx), 1) // P))
        idxs.append(idx)
    nk = nkc * P
    for b in range(x.shape[0]):
        idx = idxs[b]
        n = len(idx)
        pad = np.zeros(nk - n, dtype=np.int64)
        idxs[b] = np.concatenate([idx, pad])
        keeps.append(np.concatenate([np.ones(n, f32), np.zeros(nk - n, f32)]))

    def wlayout(w, kc):
        w = np.asarray(w, f32)
        return np.ascontiguousarray(
            w.reshape(kc, P, w.shape[1]).transpose(1, 0, 2)).astype(bf16)

    shared = {
        "wq": wlayout(wq, KC), "wk": wlayout(wk, KC), "wv": wlayout(wv, KC),
        "wo": wlayout(wo, KC), "w1": wlayout(w1, KC),
        "w2": wlayout(w2, FF // P),
        "ln1w": np.asarray(ln1_w, f32).reshape(1, D),
        "ln1b": np.asarray(ln1_b, f32).reshape(1, D),
        "ln2w": np.asarray(ln2_w, f32).reshape(1, D),
        "ln2b": np.asarray(ln2_b, f32).reshape(1, D),
    }

    def tposed(a, nchunks):  # [n*P, w] -> [P, n, w]
        return np.ascontiguousarray(
            a.reshape(nchunks, P, a.shape[1]).transpose(1, 0, 2))

    in_maps = []
    for c in range(NCORES):
        b, half = divmod(c, 2)
        idx, keep = idxs[b], keeps[b]
        m = dict(shared)
        m["xg"] = np.ascontiguousarray(x[b][idx]).astype(bf16)
        m["xq"] = np.ascontiguousarray(x[b, half * TL:(half + 1) * TL])
        mmp = keep.reshape(nkc, P).T  # [P, nkc]
        m["mmz"] = np.ascontiguousarray(
            np.stack([mmp, np.zeros_like(mmp)], axis=-1))
        trig = np.concatenate([
            tposed(ci[idx], nkc), tposed(si[idx], nkc),
            tposed(ci[half * TL:(half + 1) * TL], 4),
            tposed(si[half * TL:(half + 1) * TL], 4)], axis=1)
        m["trig"] = trig.astype(bf16)
        in_maps.append(m)
    return in_maps, nkc


def kernel(x, mask, freqs_cos, freqs_sin, wq, wk, wv, wo, w1, w2,
           ln1_w, ln1_b, ln2_w, ln2_b, _trace=False, _sim=False):
    from concourse.bass_utils import run_bass_kernel_spmd

    apply_ln1 = not (np.all(np.asarray(ln1_w) == 1.0)
                     and np.all(np.asarray(ln1_b) == 0.0))
    apply_ln2 = not (np.all(np.asarray(ln2_w) == 1.0)
                     and np.all(np.asarray(ln2_b) == 0.0))
    in_maps, nkc = _prep_inputs(x, mask, freqs_cos, freqs_sin, wq, wk, wv, wo,
                                w1, w2, ln1_w, ln1_b, ln2_w, ln2_b)
    nc = _get_program(nkc, apply_ln1, apply_ln2, sim_compat=_sim)

    if _sim:
        from concourse.bass_interp import CoreSim
        sim = CoreSim(nc, trace=False)
        for k, v in in_maps[0].items():
            sim.tensor(k)[:] = v
        sim.simulate(check_with_hw=False)
        B, S = x.shape[0], x.shape[1]
        full = np.empty((B, S, D), np.float32)
        full[0, :TL] = np.array(sim.tensor("out"))
        return full

    res = run_bass_kernel_spmd(nc, in_maps, core_ids=list(range(NCORES)),
                               trace=_trace)
    B, S = x.shape[0], x.shape[1]
    full = np.empty((B, S, D), np.float32)
    for c in range(NCORES):
        b, half = divmod(c, 2)
        full[b, half * TL:(half + 1) * TL] = res.results[c]["out"]
    if _trace:
        return full, res
    return full


# revision 23
# speedup vs baseline: 1.0064x; 1.0064x over previous
"""Trainium2 Bass kernel for a dense transformer block (nn_Block_3453153706485).

B=4, S=1024, D=1024, H=16 heads (hd=64), FF=4096, fp32 I/O.
Sharding: 8 cores; core c owns (batch b=c//2, token half c%2) -> 512 query
tokens.

Key optimization: ~50% of keys are masked out by the key-padding mask and
contribute nothing.  The HOST gathers the unmasked key tokens per batch
(padded to a multiple of 128 with duplicates of token 0 carrying keep=0) so
k/v projections, scores, exp and p@v run only on ~5/8 of the sequence.

All matmuls bf16 (residual fp32).  Transposes run on the DMA XBAR
(dma_start_transpose, sync queue).  LayerNorm rstd = exp(-0.5*ln(var+eps))
so the whole pre-MLP kernel stays on one ACT table set (natural_log_exp).
Softmax: p@v head pairs accumulate into one 2-bank psum with a keep-mask
row giving the denominator Z; Z rows are batch-reciprocal'd, staged through
DRAM, broadcast, and attnT is normalized in place in bf16.
"""

from contextlib import ExitStack

import ml_dtypes
import numpy as np

import concourse.bass as bass
import concourse.tile as tile
from concourse import bacc, mybir

# Make Exp and Ln resolve to the single combined ACT table set so the
# whole pre-MLP kernel needs one table load (the greedy per-function set
# picker would otherwise thrash natural_log <-> exp_and_others per LN).
import concourse.hw_specs as _hw_specs
import concourse.bacc as _bacc_mod
import concourse.bass_interp as _interp_mod

_orig_gat = _hw_specs.get_activation_tables


def _gat_combined(arch):
    _AF = mybir.ActivationFunctionType
    tables = {}
    for name, fns in _orig_gat(arch).items():
        fns = set(fns)
        if name != "natural_log_exp_and_others":
            fns.discard(_AF.Exp)
            fns.discard(_AF.Ln)
        tables[name] = fns
    return tables


_bacc_mod.get_activation_tables = _gat_combined
_interp_mod.get_activation_tables = _gat_combined

F32 = mybir.dt.float32
BF16 = mybir.dt.bfloat16
AF = mybir.ActivationFunctionType
OP = mybir.AluOpType

P = 128
D = 1024
H = 16
HD = 64
FF = 4096
TL = 512           # local (query) tokens per core
KC = D // P        # 8
NCORES = 8
EPS = 1e-5


def _swap_pairs(ap4):
    """View with the two elements of each innermost [step,2] pair swapped."""
    st = ap4.ap[-1][0]
    return bass.AP(
        tensor=ap4.tensor,
        offset=ap4.offset + st,
        ap=list(ap4.ap[:-1]) + [[-st, 2]],
    )


def build_program(nkc: int, apply_ln1: bool, apply_ln2: bool,
                  sim_compat: bool = False):
    nc = bacc.Bacc("TRN2", target_bir_lowering=False, debug=False)
    nk = nkc * P
    NQ = TL // P       # 4

    xg = nc.dram_tensor("xg", [nk, D], BF16, kind="ExternalInput").ap()
    xq = nc.dram_tensor("xq", [TL, D], F32, kind="ExternalInput").ap()
    mmz_d = nc.dram_tensor("mmz", [P, nkc, 2], F32, kind="ExternalInput").ap()
    trig_d = nc.dram_tensor("trig", [P, 2 * nkc + 8, HD], BF16,
                            kind="ExternalInput").ap()
    wq_d = nc.dram_tensor("wq", [P, KC, D], BF16, kind="ExternalInput").ap()
    wk_d = nc.dram_tensor("wk", [P, KC, D], BF16, kind="ExternalInput").ap()
    wv_d = nc.dram_tensor("wv", [P, KC, D], BF16, kind="ExternalInput").ap()
    wo_d = nc.dram_tensor("wo", [P, KC, D], BF16, kind="ExternalInput").ap()
    w1_d = nc.dram_tensor("w1", [P, KC, FF], BF16, kind="ExternalInput").ap()
    w2_d = nc.dram_tensor("w2", [P, FF // P, D], BF16, kind="ExternalInput").ap()
    ln_d = {nm: nc.dram_tensor(nm, [1, D], F32, kind="ExternalInput").ap()
            for nm in ("ln1w", "ln1b", "ln2w", "ln2b")}
    out_d = nc.dram_tensor("out", [TL, D], F32, kind="ExternalOutput").ap()
    zs_d = nc.dram_tensor("zstage", [H, TL], BF16, kind="Internal").ap()

    gelu_f = AF.Identity if sim_compat else AF.Gelu

    with tile.TileContext(nc) as tc:
        es0 = ExitStack()

        cons = es0.enter_context(tc.tile_pool(name="cons", bufs=1))
        work = es0.enter_context(tc.tile_pool(name="work", bufs=2))

        # ---- x loads first (they gate everything) ----
        pool_hT = tc.alloc_tile_pool(name="p_hT", bufs=1, side="right")
        pool_xg = tc.alloc_tile_pool(name="p_xg", bufs=1, side="right")
        xts = []
        for ti in range(nkc):
            xt = pool_xg.tile([P, D], BF16, name=f"xt{ti}")
            nc.sync.dma_start(xt[:], xg[ti * P:(ti + 1) * P, :])
            xts.append(xt)
        # ---------------- constants ----------------
        eps_t = cons.tile([P, 1], F32, name="eps_t")
        nc.vector.memset(eps_t[:], EPS)

        pool_kT = tc.alloc_tile_pool(name="p_kT", bufs=1)
        pool_v1 = tc.alloc_tile_pool(name="p_v1", bufs=1)
        pool_qT = tc.alloc_tile_pool(name="p_qT", bufs=1)

        # wk halves on the fast HWDGE sync queue (first compute needs them);
        # trig/mmz then xq on gpsimd (slow SWDGE, needed later)
        pool_wB = tc.alloc_tile_pool(name="p_wB", bufs=1)
        wkh, wvh, wqh = [], [], []
        for nh in range(2):
            t = pool_wB.tile([P, KC, 512], BF16, name=f"wk{nh}")
            nc.sync.dma_start(t[:], wk_d[:, :, nh * 512:(nh + 1) * 512])
            wkh.append(t)

        trig = cons.tile([P, 2 * nkc + 8, HD], BF16, name="trig_sb")
        nc.gpsimd.dma_start(trig[:], trig_d)
        cik = trig[:, 0:nkc, :]
        sik = trig[:, nkc:2 * nkc, :]
        ciq = trig[:, 2 * nkc:2 * nkc + 4, :]
        siq = trig[:, 2 * nkc + 4:2 * nkc + 8, :]
        mmz = cons.tile([P, nkc, 2], F32, name="mmz_sb")
        nc.gpsimd.dma_start(mmz[:], mmz_d)

        xqs = []
        for ti in range(NQ):
            xtq = work.tile([P, D], F32, tag="xtq", bufs=NQ, name=f"xtq{ti}")
            nc.gpsimd.dma_start(xtq[:], xq[ti * P:(ti + 1) * P, :])
            xqs.append(xtq)

        ln_bc = {}
        for nm, need in (("ln1w", apply_ln1), ("ln1b", apply_ln1),
                         ("ln2w", apply_ln2), ("ln2b", apply_ln2)):
            if need:
                t = cons.tile([P, D], F32, name=f"{nm}_bc")
                src = bass.AP(tensor=ln_d[nm].tensor, offset=ln_d[nm].offset,
                              ap=[[0, P], [1, D]])
                nc.gpsimd.dma_start(t[:], src)
                ln_bc[nm] = t

        def layernorm(src_ap, dst_ap, wkey, bkey, applied):
            """src [P, D] (any dtype) -> dst [P, D] bf16 normalized.

            rstd = exp(-0.5*ln(var+eps)): stays on the natural_log_exp ACT
            table set, so no table switches between LN / softmax-exp.
            """
            stats = work.tile([P, 2, 6], F32, tag="stats", bufs=4, name="st")
            nc.vector.bn_stats(stats[:, 0, :], src_ap[:, 0:512])
            nc.vector.bn_stats(stats[:, 1, :], src_ap[:, 512:1024])
            mv = work.tile([P, 2], F32, tag="mv", bufs=4, name="mv")
            nc.vector.bn_aggr(mv[:], stats[:])
            std = work.tile([P, 1], F32, tag="std", bufs=4, name="std")
            nc.scalar.activation(std[:], mv[:, 1:2], AF.Sqrt, bias=eps_t[:])
            rstd = work.tile([P, 1], F32, tag="rstd", bufs=4, name="rstd")
            nc.vector.reciprocal(rstd[:], std[:])
            nc.vector.tensor_scalar(dst_ap, src_ap, mv[:, 0:1], rstd[:],
                                    OP.subtract, OP.mult)
            if applied:
                nc.vector.tensor_tensor(dst_ap, dst_ap, ln_bc[wkey][:], OP.mult)
                nc.vector.tensor_tensor(dst_ap, dst_ap, ln_bc[bkey][:], OP.add)

        def rope(krb, ci, si, ti, dst_ap, nh8):
            """RoPE a [P, nh8*64] bf16 tile -> dst bf16 sbuf."""
            w = nh8 * HD
            kb_h = krb.rearrange("p (h i) -> p h i", h=nh8)
            ci_b = ci[:, ti, None, :].to_broadcast((P, nh8, HD))
            p1 = work.tile([P, w], BF16, tag=f"p1_{nh8}", bufs=2, name="p1")
            nc.vector.tensor_tensor(p1.rearrange("p (h i) -> p h i", h=nh8),
                                    kb_h, ci_b, OP.mult)
            kb_sw = _swap_pairs(krb.rearrange("p (h i two) -> p h i two",
                                              h=nh8, two=2))
            si_b = (si[:, ti, None, :].to_broadcast((P, nh8, HD))
                    .rearrange("p h (i two) -> p h i two", two=2))
            p2 = work.tile([P, w], BF16, tag=f"p2_{nh8}", bufs=2, name="p2")
            nc.vector.tensor_tensor(
                p2.rearrange("p (h i two) -> p h i two", h=nh8, two=2),
                kb_sw, si_b, OP.mult)
            nc.vector.tensor_tensor(dst_ap, p1[:], p2[:], OP.add)

        def proj_group(dst_ps, lhs_base, wtile):
            for kc in range(KC):
                nc.tensor.matmul(dst_ps, lhsT=lhs_base[:, kc, :],
                                 rhs=wtile[:, kc, :],
                                 start=(kc == 0), stop=(kc == KC - 1))

        psB = tc.alloc_tile_pool(name="psB", bufs=1, space="PSUM")

        def ps512(nm):
            return psB.tile([P, 512], F32, tag="ps512", bufs=4, name=nm)

        # ========== stage A: LN1 of gathered key tokens -> hT ==========
        hT = pool_hT.tile([P, nkc, KC, P], BF16, name="hT")
        for ti in range(nkc):
            h = work.tile([P, D], BF16, tag="h", bufs=3, name=f"h{ti}")
            layernorm(xts[ti][:], h[:], "ln1w", "ln1b", apply_ln1)
            nc.sync.dma_start_transpose(hT[:, ti, :, :], h[:])
        pool_xg.release()

        # wv/wq halves on the scalar HWDGE queue, after stage A's LN work
        for nm, lst, srcd in (("wv", wvh, wv_d), ("wq", wqh, wq_d)):
            for nh in range(2):
                t = pool_wB.tile([P, KC, 512], BF16, name=f"{nm}{nh}")
                nc.scalar.dma_start(t[:], srcd[:, :, nh * 512:(nh + 1) * 512])
                lst.append(t)

        # ========== stage B: k, v (gathered keys), q (local) ==========
        kT = pool_kT.tile([P, nkc, KC, P], BF16, name="kT")
        for ti in range(nkc):
            krb = work.tile([P, D], BF16, tag="krb", bufs=2, name=f"krb{ti}")
            for nh in range(2):
                ps = ps512(f"kps{nh}_{ti}")
                proj_group(ps, hT[:, ti, :, :], wkh[nh])
                nc.scalar.copy(krb[:, nh * 512:(nh + 1) * 512], ps[:])
            kr = work.tile([P, D], BF16, tag="kr", bufs=2, name=f"kr{ti}")
            rope(krb[:], cik, sik, ti, kr[:], 16)
            nc.sync.dma_start_transpose(kT[:, ti, :, :], kr[:])

        v1 = pool_v1.tile([P, nkc, H, 66], BF16, name="v1")
        # v1 mask columns: col 64 = keep flag (1/0), col 65 = 0 (pad)
        for ti in range(nkc):
            nc.vector.tensor_copy(v1[:, ti, :, 64:66],
                                  mmz[:, ti, None, :].to_broadcast((P, H, 2)))
        for nh in range(2):
            for ti in range(nkc):
                ps = ps512(f"vps{nh}_{ti}")
                proj_group(ps, hT[:, ti, :, :], wvh[nh])
                # eviction with per-token keep-scale on the ACT engine
                nc.scalar.activation(
                    v1[:, ti, nh * 8:(nh + 1) * 8, 0:64],
                    ps.rearrange("p (h d) -> p h d", h=8),
                    AF.Copy, scale=mmz[:, ti, 0:1])

        # q path: nh-outer so scores for head-pairs 0-3 can start while the
        # nh=1 projections still run
        qT = pool_qT.tile([P, NQ, KC, P], BF16, name="qT")
        hqTs = []
        for ti in range(NQ):
            hq = work.tile([P, D], BF16, tag="h", bufs=3, name=f"hq{ti}")
            layernorm(xqs[ti][:], hq[:], "ln1w", "ln1b", apply_ln1)
            hqT = work.tile([P, KC, P], BF16, tag="hqT", bufs=NQ,
                            name=f"hqT{ti}")
            nc.sync.dma_start_transpose(hqT[:], hq[:])
            hqTs.append(hqT)
        for nh in range(2):
            for ti in range(NQ):
                ps = ps512(f"qps{nh}_{ti}")
                proj_group(ps, hqTs[ti], wqh[nh])
                qrb = work.tile([P, 512], BF16, tag="qrb", bufs=2,
                                name=f"qrb{nh}_{ti}")
                nc.scalar.copy(qrb[:], ps[:])
                qr = work.tile([P, 512], BF16, tag="qr", bufs=3,
                               name=f"qr{nh}_{ti}")
                rope(qrb[:], ciq, siq, ti, qr[:], 8)
                nc.sync.dma_start_transpose(
                    qT[:, ti, nh * 4:(nh + 1) * 4, :], qr[:])

        pool_wB.release()
        pool_hT.release()
        psB.release()

        # ========== stage C: attention per head-pair ==========
        pool_wD = tc.alloc_tile_pool(name="p_wD", bufs=1, side="right")
        woh = []
        for nh in range(2):
            t = pool_wD.tile([P, KC, 512], BF16, name=f"wo{nh}")
            nc.gpsimd.dma_start(t[:], wo_d[:, :, nh * 512:(nh + 1) * 512])
            woh.append(t)

        psC = tc.alloc_tile_pool(name="psC", bufs=1, space="PSUM")
        pool_at = tc.alloc_tile_pool(name="p_at", bufs=1, side="right")
        pool_pT = tc.alloc_tile_pool(name="p_pT", bufs=2, side="right")
        pool_z = tc.alloc_tile_pool(name="p_z", bufs=1, side="right")
        pool_zbc = tc.alloc_tile_pool(name="p_zbc", bufs=1, side="right")
        attnT = pool_at.tile([P, KC, TL], BF16, name="attnT")
        ztabs = {}

        def extract_z(pj, pvv):
            # Z rows -> ztab rows {0,32,64,96} (DVE)
            for eo in range(2):
                hh = 2 * pj + eo
                hg, hi = divmod(hh, 4)
                if hi == 0:
                    ztabs[hg] = pool_z.tile([P, TL], F32, tag="ztab", bufs=2,
                                            name=f"ztab{hg}")
                    nc.vector.memset(ztabs[hg][:], 1.0)
                nc.vector.tensor_copy(ztabs[hg][32 * hi:32 * hi + 1, :],
                                      pvv[64:65, eo * 512:(eo + 1) * 512])

        def recip_stage(hg):
            # batched reciprocal for 4 heads (rows 0/32/64/96); cast to
            # bf16; stage through DRAM
            ztab = ztabs.pop(hg)
            nc.vector.reciprocal(ztab[:], ztab[:])
            zcast = pool_z.tile([P, TL], BF16, tag="zcast", bufs=2,
                                name=f"zcast{hg}")
            nc.vector.tensor_copy(zcast[:], ztab[:])
            for hi in range(4):
                hh = hg * 4 + hi
                dst = bass.AP(tensor=zs_d.tensor,
                              offset=zs_d.offset + hh * TL,
                              ap=[[TL, 1], [1, TL]])
                nc.sync.dma_start(dst, zcast[32 * hi:32 * hi + 1, :])

        def evict_attn(pj, pvv):
            # unnormalized attnT evict (DVE, bf16 cast)
            for eo in range(2):
                pb = 64 * eo
                nc.vector.tensor_copy(attnT[pb:pb + 64, pj, :],
                                      pvv[0:64, eo * 512:(eo + 1) * 512])

        def emit_normalize(hg):
            # broadcast 1/Z for 4 heads in one DMA; in-place bf16 normalize
            zbc = pool_zbc.tile([P, 4, TL], BF16, tag="zbc", bufs=2,
                                name=f"zbc{hg}")
            src = bass.AP(tensor=zs_d.tensor, offset=zs_d.offset + hg * 4 * TL,
                          ap=[[0, P], [TL, 4], [1, TL]])
            nc.sync.dma_start(zbc[:], src)
            for hi in range(4):
                hh = hg * 4 + hi
                pj, eo = divmod(hh, 2)
                pb = 64 * eo
                nc.vector.tensor_tensor(attnT[pb:pb + 64, pj, :],
                                        attnT[pb:pb + 64, pj, :],
                                        zbc[pb:pb + 64, hi, :], OP.mult)

        prev = None
        for j in range(H // 2 + 1):
            last = j == H // 2
            if not last:
                pt = pool_pT.tile([P, nkc, 2, 512], BF16, tag="pT",
                                  name=f"pT{j}")
            for skc in range(nkc):
                if not last:
                    pss = psC.tile([P, 1024], F32, tag="sc", bufs=2,
                                   name=f"scps{j}_{skc}")
                    for eo in range(2):
                        pb = 64 * eo
                        nc.tensor.matmul(
                            pss[:, eo * 512:(eo + 1) * 512],
                            lhsT=kT[pb:pb + 64, skc, j, :],
                            rhs=qT[pb:pb + 64, :, j, :],
                            start=True, stop=True)
                    nc.scalar.activation(
                        pt[:, skc, :, :].rearrange("p a b -> p (a b)"),
                        pss[:], AF.Exp, scale=0.125)
                if prev is not None:
                    pj, ppt, pvv = prev
                    for eo in range(2):
                        nc.tensor.matmul(
                            pvv[0:66, eo * 512:(eo + 1) * 512],
                            lhsT=v1[:, skc, 2 * pj + eo, :],
                            rhs=ppt[:, skc, eo, :],
                            start=(skc == 0), stop=(skc == nkc - 1))
            if prev is not None:
                pj = prev[0]
                extract_z(pj, prev[2])
                if pj % 2 == 1:
                    recip_stage(pj // 2)
                evict_attn(pj, prev[2])
                if pj % 2 == 1:
                    emit_normalize(pj // 2)
            if not last:
                prev = (j, pt, psC.tile([P, 1024], F32, tag="pv", bufs=2,
                                        name=f"pv{j}"))

        pool_zbc.release()
        pool_z.release()
        pool_pT.release()
        pool_qT.release()
        pool_v1.release()
        pool_kT.release()
        psC.release()

        # ========== stage D: wo + residual -> xres; LN2 -> h2T ==========
        psD = tc.alloc_tile_pool(name="psD", bufs=1, space="PSUM")
        pool_res = tc.alloc_tile_pool(name="p_res", bufs=1)
        xres = pool_res.tile([P, NQ, D], F32, name="xres")
        h2T = pool_res.tile([P, NQ, KC, P], BF16, name="h2T")
        pool_w1 = tc.alloc_tile_pool(name="p_w1", bufs=2)

        def psDt(nm):
            return psD.tile([P, 512], F32, tag="wops", bufs=2, name=nm)

        # wo split: heads 0-11 (kc 0-5) are normalized well before heads
        # 12-15 (kc 6-7); run the early part as soon as psum frees, then
        # only the 2-chunk remainder waits on the last softmax group
        for tc4 in range(NQ):
            for nh in range(2):
                ps = psDt(f"woA{tc4}_{nh}")
                for kc in range(6):
                    nc.tensor.matmul(
                        ps[:], lhsT=attnT[:, kc, tc4 * P:(tc4 + 1) * P],
                        rhs=woh[nh][:, kc, :],
                        start=(kc == 0), stop=(kc == 5))
                nc.vector.tensor_add(xres[:, tc4, nh * 512:(nh + 1) * 512],
                                     ps[:],
                                     xqs[tc4][:, nh * 512:(nh + 1) * 512])
        for tc4 in range(NQ):
            for nh in range(2):
                ps = psDt(f"woB{tc4}_{nh}")
                for kc in range(6, KC):
                    nc.tensor.matmul(
                        ps[:], lhsT=attnT[:, kc, tc4 * P:(tc4 + 1) * P],
                        rhs=woh[nh][:, kc, :],
                        start=(kc == 6), stop=(kc == KC - 1))
                nc.vector.tensor_add(xres[:, tc4, nh * 512:(nh + 1) * 512],
                                     ps[:],
                                     xres[:, tc4, nh * 512:(nh + 1) * 512])
            h2 = work.tile([P, D], BF16, tag="h", bufs=3, name=f"h2{tc4}")
            layernorm(xres[:, tc4, :], h2[:], "ln2w", "ln2b", apply_ln2)
            nc.sync.dma_start_transpose(h2T[:, tc4, :, :], h2[:])

        pool_at.release()
        pool_wD.release()
        psD.release()
        psE = tc.alloc_tile_pool(name="psE", bufs=1, space="PSUM")

        # ========== stage E: MLP ==========
        pool_g1 = tc.alloc_tile_pool(name="p_g1", bufs=1, side="right")
        pool_w2 = tc.alloc_tile_pool(name="p_w2", bufs=2, side="right")
        g1 = pool_g1.tile([P, FF // P, TL], BF16, name="g1")

        for fg in range(FF // 1024):
            w1c = pool_w1.tile([P, KC, 1024], BF16, tag="w1c", name=f"w1c{fg}")
            nc.scalar.dma_start(w1c[:], w1_d[:, :, fg * 1024:(fg + 1) * 1024])
            for j in range(8):
                ps = psE.tile([P, 512], F32, tag="m1ps", bufs=4,
                              name=f"m1ps{fg}_{j}")
                # split over tc-halves so the first MMs start before the
                # last h2T tile lands
                for c2 in range(2):
                    for kc in range(KC):
                        nc.tensor.matmul(
                            ps[:, c2 * 256:(c2 + 1) * 256],
                            lhsT=w1c[:, kc, j * P:(j + 1) * P],
                            rhs=h2T[:, 2 * c2:2 * c2 + 2, kc, :],
                            start=(kc == 0), stop=(kc == KC - 1))
                nc.scalar.activation(g1[:, fg * 8 + j, :], ps[:], gelu_f)

        ots = []
        for tc4 in range(NQ):
            ots.append(work.tile([P, D], F32, tag="osb", bufs=NQ,
                                 name=f"ot{tc4}"))
        for nh in range(2):
            psos = [psE.tile([P, 512], F32, tag="m2ps", bufs=4,
                             name=f"m2ps{nh}_{tc4}") for tc4 in range(NQ)]
            for kg in range(4):
                w2c = pool_w2.tile([P, 8, 512], BF16, tag="w2c",
                                   name=f"w2c{nh}_{kg}")
                nc.scalar.dma_start(
                    w2c[:], w2_d[:, kg * 8:(kg + 1) * 8,
                                 nh * 512:(nh + 1) * 512])
                for tc4 in range(NQ):
                    for kc in range(8):
                        nc.tensor.matmul(
                            psos[tc4],
                            lhsT=g1[:, kg * 8 + kc, tc4 * P:(tc4 + 1) * P],
                            rhs=w2c[:, kc, :],
                            start=(kg == 0 and kc == 0),
                            stop=(kg == 3 and kc == 7))
            for tc4 in range(NQ):
                nc.vector.tensor_add(
                    ots[tc4][:, nh * 512:(nh + 1) * 512], psos[tc4][:],
                    xres[:, tc4, nh * 512:(nh + 1) * 512])
        for tc4 in range(NQ):
            nc.sync.dma_start(out_d[tc4 * P:(tc4 + 1) * P, :], ots[tc4][:])

        pool_w2.release()
        pool_g1.release()
        pool_w1.release()
        pool_res.release()
        psE.release()
        es0.close()

    nc.compile()
    return nc


# ---------------------------------------------------------------------------
# Host side
# ---------------------------------------------------------------------------

_PROGRAM_CACHE = {}


def _get_program(nkc, apply_ln1, apply_ln2, sim_compat=False):
    key = (nkc, apply_ln1, apply_ln2, sim_compat)
    if key not in _PROGRAM_CACHE:
        _PROGRAM_CACHE[key] = build_program(*key)
    return _PROGRAM_CACHE[key]


def _prep_inputs(x, mask, freqs_cos, freqs_sin, wq, wk, wv, wo, w1, w2,
                 ln1_w, ln1_b, ln2_w, ln2_b):
    """Build the 8 per-core input dicts.  Returns (in_maps, nkc)."""
    f32 = np.float32
    bf16 = ml_dtypes.bfloat16
    x = np.asarray(x, f32)
    mask = np.asarray(mask)
    cos = np.asarray(freqs_cos, f32)
    sin = np.asarray(freqs_sin, f32)
    S = x.shape[1]

    ci = np.empty((S, HD), f32)
    ci[:, 0::2] = cos
    ci[:, 1::2] = cos
    si = np.empty((S, HD), f32)
    si[:, 0::2] = -sin
    si[:, 1::2] = sin

    # gather unmasked key tokens per batch, pad to a common multiple of 128
    idxs, keeps = [], []
    nkc = 1
    for b in range(x.shape[0]):
        idx = np.nonzero(~mask[b])[0]
        nkc = max(nkc, -(-max(len(idx), 1) // P))
        idxs.append(idx)
    nk = nkc * P
    for b in range(x.shape[0]):
        idx = idxs[b]
        n = len(idx)
        pad = np.zeros(nk - n, dtype=np.int64)
        idxs[b] = np.concatenate([idx, pad])
        keeps.append(np.concatenate([np.ones(n, f32), np.zeros(nk - n, f32)]))

    def wlayout(w, kc):
        w = np.asarray(w, f32)
        return np.ascontiguousarray(
            w.reshape(kc, P, w.shape[1]).transpose(1, 0, 2)).astype(bf16)

    shared = {
        "wq": wlayout(wq, KC), "wk": wlayout(wk, KC), "wv": wlayout(wv, KC),
        "wo": wlayout(wo, KC), "w1": wlayout(w1, KC),
        "w2": wlayout(w2, FF // P),
        "ln1w": np.asarray(ln1_w, f32).reshape(1, D),
        "ln1b": np.asarray(ln1_b, f32).reshape(1, D),
        "ln2w": np.asarray(ln2_w, f32).reshape(1, D),
        "ln2b": np.asarray(ln2_b, f32).reshape(1, D),
    }

    def tposed(a, nchunks):  # [n*P, w] -> [P, n, w]
        return np.ascontiguousarray(
            a.reshape(nchunks, P, a.shape[1]).transpose(1, 0, 2))

    in_maps = []
    for c in range(NCORES):
        b, half = divmod(c, 2)
        idx, keep = idxs[b], keeps[b]
        m = dict(shared)
        m["xg"] = np.ascontiguousarray(x[b][idx]).astype(bf16)
        m["xq"] = np.ascontiguousarray(x[b, half * TL:(half + 1) * TL])
        mmp = keep.reshape(nkc, P).T  # [P, nkc]
        m["mmz"] = np.ascontiguousarray(
            np.stack([mmp, np.zeros_like(mmp)], axis=-1))
        trig = np.concatenate([
            tposed(ci[idx], nkc), tposed(si[idx], nkc),
            tposed(ci[half * TL:(half + 1) * TL], 4),
            tposed(si[half * TL:(half + 1) * TL], 4)], axis=1)
        m["trig"] = trig.astype(bf16)
        in_maps.append(m)
    return in_maps, nkc


def kernel(x, mask, freqs_cos, freqs_sin, wq, wk, wv, wo, w1, w2,
           ln1_w, ln1_b, ln2_w, ln2_b, _trace=False, _sim=False):
    from concourse.bass_utils import run_bass_kernel_spmd

    apply_ln1 = not (np.all(np.asarray(ln1_w) == 1.0)
                     and np.all(np.asarray(ln1_b) == 0.0))
    apply_ln2 = not (np.all(np.asarray(ln2_w) == 1.0)
                     and np.all(np.asarray(ln2_b) == 0.0))
    in_maps, nkc = _prep_inputs(x, mask, freqs_cos, freqs_sin, wq, wk, wv, wo,
                                w1, w2, ln1_w, ln1_b, ln2_w, ln2_b)
    nc = _get_program(nkc, apply_ln1, apply_ln2, sim_compat=_sim)

    if _sim:
        from concourse.bass_interp import CoreSim
        sim = CoreSim(nc, trace=False)
        for k, v in in_maps[0].items():
            sim.tensor(k)[:] = v
        sim.simulate(check_with_hw=False)
        B, S = x.shape[0], x.shape[1]
        full = np.empty((B, S, D), np.float32)
        full[0, :TL] = np.array(sim.tensor("out"))
        return full

    res = run_bass_kernel_spmd(nc, in_maps, core_ids=list(range(NCORES)),
                               trace=_trace)
    B, S = x.shape[0], x.shape[1]
    full = np.empty((B, S, D), np.float32)
    for c in range(NCORES):
        b, half = divmod(c, 2)
        full[b, half * TL:(half + 1) * TL] = res.results[c]["out"]
    if _trace:
        return full, res
    return full


# revision 24
# speedup vs baseline: 1.0410x; 1.0344x over previous
"""Trainium2 Bass kernel for a dense transformer block (nn_Block_3453153706485).

B=4, S=1024, D=1024, H=16 heads (hd=64), FF=4096, fp32 I/O.
Sharding: 8 cores; core c owns (batch b=c//2, token half c%2) -> 512 query
tokens.

Key optimization: ~50% of keys are masked out by the key-padding mask and
contribute nothing.  The HOST gathers the unmasked key tokens per batch
(padded to a multiple of 128 with duplicates of token 0 carrying keep=0) so
k/v projections, scores, exp and p@v run only on ~5/8 of the sequence.

All matmuls bf16 (residual fp32).  Transposes run on the DMA XBAR
(dma_start_transpose, sync queue).  LayerNorm rstd = exp(-0.5*ln(var+eps))
so the whole pre-MLP kernel stays on one ACT table set (natural_log_exp).
Softmax: p@v head pairs accumulate into one 2-bank psum with a keep-mask
row giving the denominator Z; Z rows are batch-reciprocal'd, staged through
DRAM, broadcast, and attnT is normalized in place in bf16.
"""

from contextlib import ExitStack

import ml_dtypes
import numpy as np

import concourse.bass as bass
import concourse.tile as tile
from concourse import bacc, mybir

# Make Exp and Ln resolve to the single combined ACT table set so the
# whole pre-MLP kernel needs one table load (the greedy per-function set
# picker would otherwise thrash natural_log <-> exp_and_others per LN).
import concourse.hw_specs as _hw_specs
import concourse.bacc as _bacc_mod
import concourse.bass_interp as _interp_mod

_orig_gat = _hw_specs.get_activation_tables


def _gat_combined(arch):
    _AF = mybir.ActivationFunctionType
    tables = {}
    for name, fns in _orig_gat(arch).items():
        fns = set(fns)
        if name != "natural_log_exp_and_others":
            fns.discard(_AF.Exp)
            fns.discard(_AF.Ln)
        tables[name] = fns
    return tables


_bacc_mod.get_activation_tables = _gat_combined
_interp_mod.get_activation_tables = _gat_combined

F32 = mybir.dt.float32
BF16 = mybir.dt.bfloat16
AF = mybir.ActivationFunctionType
OP = mybir.AluOpType

P = 128
D = 1024
H = 16
HD = 64
FF = 4096
TL = 512           # local (query) tokens per core
KC = D // P        # 8
NCORES = 8
EPS = 1e-5


def _swap_pairs(ap4):
    """View with the two elements of each innermost [step,2] pair swapped."""
    st = ap4.ap[-1][0]
    return bass.AP(
        tensor=ap4.tensor,
        offset=ap4.offset + st,
        ap=list(ap4.ap[:-1]) + [[-st, 2]],
    )


def build_program(nkc: int, apply_ln1: bool, apply_ln2: bool,
                  sim_compat: bool = False):
    nc = bacc.Bacc("TRN2", target_bir_lowering=False, debug=False)
    nk = nkc * P
    NQ = TL // P       # 4

    xg = nc.dram_tensor("xg", [nk, D], BF16, kind="ExternalInput").ap()
    xq = nc.dram_tensor("xq", [TL, D], F32, kind="ExternalInput").ap()
    mmz_d = nc.dram_tensor("mmz", [P, nkc, 2], F32, kind="ExternalInput").ap()
    trig_d = nc.dram_tensor("trig", [P, 2 * nkc + 8, HD], BF16,
                            kind="ExternalInput").ap()
    wq_d = nc.dram_tensor("wq", [P, KC, D], BF16, kind="ExternalInput").ap()
    wk_d = nc.dram_tensor("wk", [P, KC, D], BF16, kind="ExternalInput").ap()
    wv_d = nc.dram_tensor("wv", [P, KC, D], BF16, kind="ExternalInput").ap()
    wo_d = nc.dram_tensor("wo", [P, KC, D], BF16, kind="ExternalInput").ap()
    w1_d = nc.dram_tensor("w1", [P, KC, FF], BF16, kind="ExternalInput").ap()
    w2_d = nc.dram_tensor("w2", [P, FF // P, D], BF16, kind="ExternalInput").ap()
    ln_d = {nm: nc.dram_tensor(nm, [1, D], F32, kind="ExternalInput").ap()
            for nm in ("ln1w", "ln1b", "ln2w", "ln2b")}
    out_d = nc.dram_tensor("out", [TL, D], F32, kind="ExternalOutput").ap()
    zs_d = nc.dram_tensor("zstage", [H, TL], BF16, kind="Internal").ap()

    gelu_f = AF.Identity if sim_compat else AF.Gelu

    with tile.TileContext(nc) as tc:
        es0 = ExitStack()

        cons = es0.enter_context(tc.tile_pool(name="cons", bufs=1))
        work = es0.enter_context(tc.tile_pool(name="work", bufs=2))

        # ---- x loads first (they gate everything) ----
        pool_hT = tc.alloc_tile_pool(name="p_hT", bufs=1, side="right")
        pool_xg = tc.alloc_tile_pool(name="p_xg", bufs=1, side="right")
        xts = []
        for ti in range(nkc):
            xt = pool_xg.tile([P, D], BF16, name=f"xt{ti}")
            nc.sync.dma_start(xt[:], xg[ti * P:(ti + 1) * P, :])
            xts.append(xt)
        # ---------------- constants ----------------
        eps_t = cons.tile([P, 1], F32, name="eps_t")
        nc.vector.memset(eps_t[:], EPS)

        pool_kT = tc.alloc_tile_pool(name="p_kT", bufs=1)
        pool_v1 = tc.alloc_tile_pool(name="p_v1", bufs=1)
        pool_qT = tc.alloc_tile_pool(name="p_qT", bufs=1)

        # wk halves on the fast HWDGE sync queue (first compute needs them);
        # trig/mmz then xq on gpsimd (slow SWDGE, needed later)
        pool_wB = tc.alloc_tile_pool(name="p_wB", bufs=1)
        wkh, wvh, wqh = [], [], []
        for nh in range(2):
            t = pool_wB.tile([P, KC, 512], BF16, name=f"wk{nh}")
            nc.sync.dma_start(t[:], wk_d[:, :, nh * 512:(nh + 1) * 512])
            wkh.append(t)

        trig = cons.tile([P, 2 * nkc + 8, HD], BF16, name="trig_sb")
        nc.gpsimd.dma_start(trig[:], trig_d)
        cik = trig[:, 0:nkc, :]
        sik = trig[:, nkc:2 * nkc, :]
        ciq = trig[:, 2 * nkc:2 * nkc + 4, :]
        siq = trig[:, 2 * nkc + 4:2 * nkc + 8, :]
        mmz = cons.tile([P, nkc, 2], F32, name="mmz_sb")
        nc.gpsimd.dma_start(mmz[:], mmz_d)

        xqs = []
        for ti in range(NQ):
            xtq = work.tile([P, D], F32, tag="xtq", bufs=NQ, name=f"xtq{ti}")
            nc.gpsimd.dma_start(xtq[:], xq[ti * P:(ti + 1) * P, :])
            xqs.append(xtq)

        ln_bc = {}
        for nm, need in (("ln1w", apply_ln1), ("ln1b", apply_ln1),
                         ("ln2w", apply_ln2), ("ln2b", apply_ln2)):
            if need:
                t = cons.tile([P, D], F32, name=f"{nm}_bc")
                src = bass.AP(tensor=ln_d[nm].tensor, offset=ln_d[nm].offset,
                              ap=[[0, P], [1, D]])
                nc.gpsimd.dma_start(t[:], src)
                ln_bc[nm] = t

        def layernorm(src_ap, dst_ap, wkey, bkey, applied):
            """src [P, D] (any dtype) -> dst [P, D] bf16 normalized.

            rstd = exp(-0.5*ln(var+eps)): stays on the natural_log_exp ACT
            table set, so no table switches between LN / softmax-exp.
            """
            stats = work.tile([P, 2, 6], F32, tag="stats", bufs=4, name="st")
            nc.vector.bn_stats(stats[:, 0, :], src_ap[:, 0:512])
            nc.vector.bn_stats(stats[:, 1, :], src_ap[:, 512:1024])
            mv = work.tile([P, 2], F32, tag="mv", bufs=4, name="mv")
            nc.vector.bn_aggr(mv[:], stats[:])
            lnv = work.tile([P, 1], F32, tag="lnv", bufs=4, name="lnv")
            nc.scalar.activation(lnv[:], mv[:, 1:2], AF.Ln, bias=eps_t[:])
            rstd = work.tile([P, 1], F32, tag="rstd", bufs=4, name="rstd")
            nc.scalar.activation(rstd[:], lnv[:], AF.Exp, scale=-0.5)
            nc.vector.tensor_scalar(dst_ap, src_ap, mv[:, 0:1], rstd[:],
                                    OP.subtract, OP.mult)
            if applied:
                nc.vector.tensor_tensor(dst_ap, dst_ap, ln_bc[wkey][:], OP.mult)
                nc.vector.tensor_tensor(dst_ap, dst_ap, ln_bc[bkey][:], OP.add)

        def rope(krb, ci, si, ti, dst_ap, nh8):
            """RoPE a [P, nh8*64] bf16 tile -> dst bf16 sbuf."""
            w = nh8 * HD
            kb_h = krb.rearrange("p (h i) -> p h i", h=nh8)
            ci_b = ci[:, ti, None, :].to_broadcast((P, nh8, HD))
            p1 = work.tile([P, w], BF16, tag=f"p1_{nh8}", bufs=2, name="p1")
            nc.vector.tensor_tensor(p1.rearrange("p (h i) -> p h i", h=nh8),
                                    kb_h, ci_b, OP.mult)
            kb_sw = _swap_pairs(krb.rearrange("p (h i two) -> p h i two",
                                              h=nh8, two=2))
            si_b = (si[:, ti, None, :].to_broadcast((P, nh8, HD))
                    .rearrange("p h (i two) -> p h i two", two=2))
            p2 = work.tile([P, w], BF16, tag=f"p2_{nh8}", bufs=2, name="p2")
            nc.vector.tensor_tensor(
                p2.rearrange("p (h i two) -> p h i two", h=nh8, two=2),
                kb_sw, si_b, OP.mult)
            nc.vector.tensor_tensor(dst_ap, p1[:], p2[:], OP.add)

        def proj_group(dst_ps, lhs_base, wtile):
            for kc in range(KC):
                nc.tensor.matmul(dst_ps, lhsT=lhs_base[:, kc, :],
                                 rhs=wtile[:, kc, :],
                                 start=(kc == 0), stop=(kc == KC - 1))

        psB = tc.alloc_tile_pool(name="psB", bufs=1, space="PSUM")

        def ps512(nm):
            return psB.tile([P, 512], F32, tag="ps512", bufs=4, name=nm)

        # ========== stage A: LN1 of gathered key tokens -> hT ==========
        hT = pool_hT.tile([P, nkc, KC, P], BF16, name="hT")
        for ti in range(nkc):
            h = work.tile([P, D], BF16, tag="h", bufs=3, name=f"h{ti}")
            layernorm(xts[ti][:], h[:], "ln1w", "ln1b", apply_ln1)
            nc.sync.dma_start_transpose(hT[:, ti, :, :], h[:])
        pool_xg.release()

        # wv/wq halves on the scalar HWDGE queue, after stage A's LN work
        for nm, lst, srcd in (("wv", wvh, wv_d), ("wq", wqh, wq_d)):
            for nh in range(2):
                t = pool_wB.tile([P, KC, 512], BF16, name=f"{nm}{nh}")
                nc.scalar.dma_start(t[:], srcd[:, :, nh * 512:(nh + 1) * 512])
                lst.append(t)

        # ========== stage B: k, v (gathered keys), q (local) ==========
        kT = pool_kT.tile([P, nkc, KC, P], BF16, name="kT")
        for ti in range(nkc):
            krb = work.tile([P, D], BF16, tag="krb", bufs=2, name=f"krb{ti}")
            for nh in range(2):
                ps = ps512(f"kps{nh}_{ti}")
                proj_group(ps, hT[:, ti, :, :], wkh[nh])
                nc.scalar.copy(krb[:, nh * 512:(nh + 1) * 512], ps[:])
            kr = work.tile([P, D], BF16, tag="kr", bufs=2, name=f"kr{ti}")
            rope(krb[:], cik, sik, ti, kr[:], 16)
            nc.sync.dma_start_transpose(kT[:, ti, :, :], kr[:])

        v1 = pool_v1.tile([P, nkc, H, 66], BF16, name="v1")
        # v1 mask columns: col 64 = keep flag (1/0), col 65 = 0 (pad)
        for ti in range(nkc):
            nc.vector.tensor_copy(v1[:, ti, :, 64:66],
                                  mmz[:, ti, None, :].to_broadcast((P, H, 2)))
        for nh in range(2):
            for ti in range(nkc):
                ps = ps512(f"vps{nh}_{ti}")
                proj_group(ps, hT[:, ti, :, :], wvh[nh])
                # eviction with per-token keep-scale on the ACT engine
                nc.scalar.activation(
                    v1[:, ti, nh * 8:(nh + 1) * 8, 0:64],
                    ps.rearrange("p (h d) -> p h d", h=8),
                    AF.Copy, scale=mmz[:, ti, 0:1])

        # q path: nh-outer so scores for head-pairs 0-3 can start while the
        # nh=1 projections still run
        qT = pool_qT.tile([P, NQ, KC, P], BF16, name="qT")
        hqTs = []
        for ti in range(NQ):
            hq = work.tile([P, D], BF16, tag="h", bufs=3, name=f"hq{ti}")
            layernorm(xqs[ti][:], hq[:], "ln1w", "ln1b", apply_ln1)
            hqT = work.tile([P, KC, P], BF16, tag="hqT", bufs=NQ,
                            name=f"hqT{ti}")
            nc.sync.dma_start_transpose(hqT[:], hq[:])
            hqTs.append(hqT)
        for nh in range(2):
            for ti in range(NQ):
                ps = ps512(f"qps{nh}_{ti}")
                proj_group(ps, hqTs[ti], wqh[nh])
                qrb = work.tile([P, 512], BF16, tag="qrb", bufs=2,
                                name=f"qrb{nh}_{ti}")
                nc.scalar.copy(qrb[:], ps[:])
                qr = work.tile([P, 512], BF16, tag="qr", bufs=3,
                               name=f"qr{nh}_{ti}")
                rope(qrb[:], ciq, siq, ti, qr[:], 8)
                nc.sync.dma_start_transpose(
                    qT[:, ti, nh * 4:(nh + 1) * 4, :], qr[:])

        pool_wB.release()
        pool_hT.release()
        psB.release()

        # ========== stage C: attention per head-pair ==========
        pool_wD = tc.alloc_tile_pool(name="p_wD", bufs=1, side="right")
        woh = []
        for nh in range(2):
            t = pool_wD.tile([P, KC, 512], BF16, name=f"wo{nh}")
            nc.gpsimd.dma_start(t[:], wo_d[:, :, nh * 512:(nh + 1) * 512])
            woh.append(t)

        psC = tc.alloc_tile_pool(name="psC", bufs=1, space="PSUM")
        pool_at = tc.alloc_tile_pool(name="p_at", bufs=1, side="right")
        pool_pT = tc.alloc_tile_pool(name="p_pT", bufs=2, side="right")
        pool_z = tc.alloc_tile_pool(name="p_z", bufs=1, side="right")
        pool_zbc = tc.alloc_tile_pool(name="p_zbc", bufs=1, side="right")
        attnT = pool_at.tile([P, KC, TL], BF16, name="attnT")
        ztabs = {}

        def extract_z(pj, pvv):
            # Z rows -> ztab rows {0,32,64,96} (DVE)
            for eo in range(2):
                hh = 2 * pj + eo
                hg, hi = divmod(hh, 4)
                if hi == 0:
                    ztabs[hg] = pool_z.tile([P, TL], F32, tag="ztab", bufs=2,
                                            name=f"ztab{hg}")
                    nc.vector.memset(ztabs[hg][:], 1.0)
                nc.vector.tensor_copy(ztabs[hg][32 * hi:32 * hi + 1, :],
                                      pvv[64:65, eo * 512:(eo + 1) * 512])

        def recip_stage(hg):
            # batched reciprocal for 4 heads (rows 0/32/64/96); cast to
            # bf16; stage through DRAM
            ztab = ztabs.pop(hg)
            nc.vector.reciprocal(ztab[:], ztab[:])
            zcast = pool_z.tile([P, TL], BF16, tag="zcast", bufs=2,
                                name=f"zcast{hg}")
            nc.vector.tensor_copy(zcast[:], ztab[:])
            for hi in range(4):
                hh = hg * 4 + hi
                dst = bass.AP(tensor=zs_d.tensor,
                              offset=zs_d.offset + hh * TL,
                              ap=[[TL, 1], [1, TL]])
                nc.sync.dma_start(dst, zcast[32 * hi:32 * hi + 1, :])

        def evict_attn(pj, pvv):
            # unnormalized attnT evict (DVE, bf16 cast)
            for eo in range(2):
                pb = 64 * eo
                nc.vector.tensor_copy(attnT[pb:pb + 64, pj, :],
                                      pvv[0:64, eo * 512:(eo + 1) * 512])

        def emit_normalize(hg):
            # broadcast 1/Z for 4 heads in one DMA; in-place bf16 normalize
            zbc = pool_zbc.tile([P, 4, TL], BF16, tag="zbc", bufs=2,
                                name=f"zbc{hg}")
            src = bass.AP(tensor=zs_d.tensor, offset=zs_d.offset + hg * 4 * TL,
                          ap=[[0, P], [TL, 4], [1, TL]])
            nc.sync.dma_start(zbc[:], src)
            for hi in range(4):
                hh = hg * 4 + hi
                pj, eo = divmod(hh, 2)
                pb = 64 * eo
                nc.vector.tensor_tensor(attnT[pb:pb + 64, pj, :],
                                        attnT[pb:pb + 64, pj, :],
                                        zbc[pb:pb + 64, hi, :], OP.mult)

        prev = None
        for j in range(H // 2 + 1):
            last = j == H // 2
            if not last:
                pt = pool_pT.tile([P, nkc, 2, 512], BF16, tag="pT",
                                  name=f"pT{j}")
            for skc in range(nkc):
                if not last:
                    pss = psC.tile([P, 1024], F32, tag="sc", bufs=2,
                                   name=f"scps{j}_{skc}")
                    for eo in range(2):
                        pb = 64 * eo
                        nc.tensor.matmul(
                            pss[:, eo * 512:(eo + 1) * 512],
                            lhsT=kT[pb:pb + 64, skc, j, :],
                            rhs=qT[pb:pb + 64, :, j, :],
                            start=True, stop=True)
                    nc.scalar.activation(
                        pt[:, skc, :, :].rearrange("p a b -> p (a b)"),
                        pss[:], AF.Exp, scale=0.125)
                if prev is not None:
                    pj, ppt, pvv = prev
                    for eo in range(2):
                        nc.tensor.matmul(
                            pvv[0:66, eo * 512:(eo + 1) * 512],
                            lhsT=v1[:, skc, 2 * pj + eo, :],
                            rhs=ppt[:, skc, eo, :],
                            start=(skc == 0), stop=(skc == nkc - 1))
            if prev is not None:
                pj = prev[0]
                extract_z(pj, prev[2])
                if pj % 2 == 1:
                    recip_stage(pj // 2)
                evict_attn(pj, prev[2])
                if pj % 2 == 1:
                    emit_normalize(pj // 2)
            if not last:
                prev = (j, pt, psC.tile([P, 1024], F32, tag="pv", bufs=2,
                                        name=f"pv{j}"))

        pool_zbc.release()
        pool_z.release()
        pool_pT.release()
        pool_qT.release()
        pool_v1.release()
        pool_kT.release()
        psC.release()

        # ========== stage D: wo + residual -> xres; LN2 -> h2T ==========
        psD = tc.alloc_tile_pool(name="psD", bufs=1, space="PSUM")
        pool_res = tc.alloc_tile_pool(name="p_res", bufs=1)
        xres = pool_res.tile([P, NQ, D], F32, name="xres")
        h2T = pool_res.tile([P, NQ, KC, P], BF16, name="h2T")
        pool_w1 = tc.alloc_tile_pool(name="p_w1", bufs=2)

        def psDt(nm):
            return psD.tile([P, 512], F32, tag="wops", bufs=2, name=nm)

        # wo split: heads 0-11 (kc 0-5) are normalized well before heads
        # 12-15 (kc 6-7); run the early part as soon as psum frees, then
        # only the 2-chunk remainder waits on the last softmax group
        for tc4 in range(NQ):
            for nh in range(2):
                ps = psDt(f"woA{tc4}_{nh}")
                for kc in range(6):
                    nc.tensor.matmul(
                        ps[:], lhsT=attnT[:, kc, tc4 * P:(tc4 + 1) * P],
                        rhs=woh[nh][:, kc, :],
                        start=(kc == 0), stop=(kc == 5))
                nc.vector.tensor_add(xres[:, tc4, nh * 512:(nh + 1) * 512],
                                     ps[:],
                                     xqs[tc4][:, nh * 512:(nh + 1) * 512])
        for tc4 in range(NQ):
            for nh in range(2):
                ps = psDt(f"woB{tc4}_{nh}")
                for kc in range(6, KC):
                    nc.tensor.matmul(
                        ps[:], lhsT=attnT[:, kc, tc4 * P:(tc4 + 1) * P],
                        rhs=woh[nh][:, kc, :],
                        start=(kc == 6), stop=(kc == KC - 1))
                nc.vector.tensor_add(xres[:, tc4, nh * 512:(nh + 1) * 512],
                                     ps[:],
                                     xres[:, tc4, nh * 512:(nh + 1) * 512])
            h2 = work.tile([P, D], BF16, tag="h", bufs=3, name=f"h2{tc4}")
            layernorm(xres[:, tc4, :], h2[:], "ln2w", "ln2b", apply_ln2)
            nc.sync.dma_start_transpose(h2T[:, tc4, :, :], h2[:])

        pool_at.release()
        pool_wD.release()
        psD.release()
        psE = tc.alloc_tile_pool(name="psE", bufs=1, space="PSUM")

        # ========== stage E: MLP ==========
        pool_g1 = tc.alloc_tile_pool(name="p_g1", bufs=1, side="right")
        pool_w2 = tc.alloc_tile_pool(name="p_w2", bufs=2, side="right")
        g1 = pool_g1.tile([P, FF // P, TL], BF16, name="g1")

        for fg in range(FF // 1024):
            w1c = pool_w1.tile([P, KC, 1024], BF16, tag="w1c", name=f"w1c{fg}")
            nc.scalar.dma_start(w1c[:], w1_d[:, :, fg * 1024:(fg + 1) * 1024])
            for j in range(8):
                ps = psE.tile([P, 512], F32, tag="m1ps", bufs=4,
                              name=f"m1ps{fg}_{j}")
                # split over tc-halves so the first MMs start before the
                # last h2T tile lands
                for c2 in range(2):
                    for kc in range(KC):
                        nc.tensor.matmul(
                            ps[:, c2 * 256:(c2 + 1) * 256],
                            lhsT=w1c[:, kc, j * P:(j + 1) * P],
                            rhs=h2T[:, 2 * c2:2 * c2 + 2, kc, :],
                            start=(kc == 0), stop=(kc == KC - 1))
                nc.scalar.activation(g1[:, fg * 8 + j, :], ps[:], gelu_f)

        ots = []
        for tc4 in range(NQ):
            ots.append(work.tile([P, D], F32, tag="osb", bufs=NQ,
                                 name=f"ot{tc4}"))
        for nh in range(2):
            psos = [psE.tile([P, 512], F32, tag="m2ps", bufs=4,
                             name=f"m2ps{nh}_{tc4}") for tc4 in range(NQ)]
            for kg in range(4):
                w2c = pool_w2.tile([P, 8, 512], BF16, tag="w2c",
                                   name=f"w2c{nh}_{kg}")
                nc.scalar.dma_start(
                    w2c[:], w2_d[:, kg * 8:(kg + 1) * 8,
                                 nh * 512:(nh + 1) * 512])
                for tc4 in range(NQ):
                    for kc in range(8):
                        nc.tensor.matmul(
                            psos[tc4],
                            lhsT=g1[:, kg * 8 + kc, tc4 * P:(tc4 + 1) * P],
                            rhs=w2c[:, kc, :],
                            start=(kg == 0 and kc == 0),
                            stop=(kg == 3 and kc == 7))
            for tc4 in range(NQ):
                nc.vector.tensor_add(
                    ots[tc4][:, nh * 512:(nh + 1) * 512], psos[tc4][:],
                    xres[:, tc4, nh * 512:(nh + 1) * 512])
        for tc4 in range(NQ):
            nc.sync.dma_start(out_d[tc4 * P:(tc4 + 1) * P, :], ots[tc4][:])

        pool_w2.release()
        pool_g1.release()
        pool_w1.release()
        pool_res.release()
        psE.release()
        es0.close()

    nc.compile()
    return nc


# ---------------------------------------------------------------------------
# Host side
# ---------------------------------------------------------------------------

_PROGRAM_CACHE = {}


def _get_program(nkc, apply_ln1, apply_ln2, sim_compat=False):
    key = (nkc, apply_ln1, apply_ln2, sim_compat)
    if key not in _PROGRAM_CACHE:
        _PROGRAM_CACHE[key] = build_program(*key)
    return _PROGRAM_CACHE[key]


def _prep_inputs(x, mask, freqs_cos, freqs_sin, wq, wk, wv, wo, w1, w2,
                 ln1_w, ln1_b, ln2_w, ln2_b):
    """Build the 8 per-core input dicts.  Returns (in_maps, nkc)."""
    f32 = np.float32
    bf16 = ml_dtypes.bfloat16
    x = np.asarray(x, f32)
    mask = np.asarray(mask)
    cos = np.asarray(freqs_cos, f32)
    sin = np.asarray(freqs_sin, f32)
    S = x.shape[1]

    ci = np.empty((S, HD), f32)
    ci[:, 0::2] = cos
    ci[:, 1::2] = cos
    si = np.empty((S, HD), f32)
    si[:, 0::2] = -sin
    si[:, 1::2] = sin

    # gather unmasked key tokens per batch, pad to a common multiple of 128
    idxs, keeps = [], []
    nkc = 1
    for b in range(x.shape[0]):
        idx = np.nonzero(~mask[b])[0]
        nkc = max(nkc, -(-max(len(idx), 1) // P))
        idxs.append(idx)
    nk = nkc * P
    for b in range(x.shape[0]):
        idx = idxs[b]
        n = len(idx)
        pad = np.zeros(nk - n, dtype=np.int64)
        idxs[b] = np.concatenate([idx, pad])
        keeps.append(np.concatenate([np.ones(n, f32), np.zeros(nk - n, f32)]))

    def wlayout(w, kc):
        w = np.asarray(w, f32)
        return np.ascontiguousarray(
            w.reshape(kc, P, w.shape[1]).transpose(1, 0, 2)).astype(bf16)

    shared = {
        "wq": wlayout(wq, KC), "wk": wlayout(wk, KC), "wv": wlayout(wv, KC),
        "wo": wlayout(wo, KC), "w1": wlayout(w1, KC),
        "w2": wlayout(w2, FF // P),
        "ln1w": np.asarray(ln1_w, f32).reshape(1, D),
        "ln1b": np.asarray(ln1_b, f32).reshape(1, D),
        "ln2w": np.asarray(ln2_w, f32).reshape(1, D),
        "ln2b": np.asarray(ln2_b, f32).reshape(1, D),
    }

    def tposed(a, nchunks):  # [n*P, w] -> [P, n, w]
        return np.ascontiguousarray(
            a.reshape(nchunks, P, a.shape[1]).transpose(1, 0, 2))

    in_maps = []
    for c in range(NCORES):
        b, half = divmod(c, 2)
        idx, keep = idxs[b], keeps[b]
        m = dict(shared)
        m["xg"] = np.ascontiguousarray(x[b][idx]).astype(bf16)
        m["xq"] = np.ascontiguousarray(x[b, half * TL:(half + 1) * TL])
        mmp = keep.reshape(nkc, P).T  # [P, nkc]
        m["mmz"] = np.ascontiguousarray(
            np.stack([mmp, np.zeros_like(mmp)], axis=-1))
        trig = np.concatenate([
            tposed(ci[idx], nkc), tposed(si[idx], nkc),
            tposed(ci[half * TL:(half + 1) * TL], 4),
            tposed(si[half * TL:(half + 1) * TL], 4)], axis=1)
        m["trig"] = trig.astype(bf16)
        in_maps.append(m)
    return in_maps, nkc


def kernel(x, mask, freqs_cos, freqs_sin, wq, wk, wv, wo, w1, w2,
           ln1_w, ln1_b, ln2_w, ln2_b, _trace=False, _sim=False):
    from concourse.bass_utils import run_bass_kernel_spmd

    apply_ln1 = not (np.all(np.asarray(ln1_w) == 1.0)
                     and np.all(np.asarray(ln1_b) == 0.0))
    apply_ln2 = not (np.all(np.asarray(ln2_w) == 1.0)
                     and np.all(np.asarray(ln2_b) == 0.0))
    in_maps, nkc = _prep_inputs(x, mask, freqs_cos, freqs_sin, wq, wk, wv, wo,
                                w1, w2, ln1_w, ln1_b, ln2_w, ln2_b)
    nc = _get_program(nkc, apply_ln1, apply_ln2, sim_compat=_sim)

    if _sim:
        from concourse.bass_interp import CoreSim
        sim = CoreSim(nc, trace=False)
        for k, v in in_maps[0].items():
            sim.tensor(k)[:] = v
        sim.simulate(check_with_hw=False)
        B, S = x.shape[0], x.shape[1]
        full = np.empty((B, S, D), np.float32)
        full[0, :TL] = np.array(sim.tensor("out"))
        return full

    res = run_bass_kernel_spmd(nc, in_maps, core_ids=list(range(NCORES)),
                               trace=_trace)
    B, S = x.shape[0], x.shape[1]
    full = np.empty((B, S, D), np.float32)
    for c in range(NCORES):
        b, half = divmod(c, 2)
        full[b, half * TL:(half + 1) * TL] = res.results[c]["out"]
    if _trace:
        return full, res
    return full


# revision 27
# speedup vs baseline: 1.0435x; 1.0024x over previous
"""Trainium2 Bass kernel for a dense transformer block (nn_Block_3453153706485).

B=4, S=1024, D=1024, H=16 heads (hd=64), FF=4096, fp32 I/O.
Sharding: 8 cores; core c owns (batch b=c//2, token half c%2) -> 512 query
tokens.

Key optimization: ~50% of keys are masked out by the key-padding mask and
contribute nothing.  The HOST gathers the unmasked key tokens per batch
(padded to a multiple of 128 with duplicates of token 0 carrying keep=0) so
k/v projections, scores, exp and p@v run only on ~5/8 of the sequence.

All matmuls bf16 (residual fp32).  Transposes run on the DMA XBAR
(dma_start_transpose, sync queue).  LayerNorm rstd = exp(-0.5*ln(var+eps))
so the whole pre-MLP kernel stays on one ACT table set (natural_log_exp).
Softmax: p@v head pairs accumulate into one 2-bank psum with a keep-mask
row giving the denominator Z; Z rows are batch-reciprocal'd, staged through
DRAM, broadcast, and attnT is normalized in place in bf16.
"""

from contextlib import ExitStack

import ml_dtypes
import numpy as np

import concourse.bass as bass
import concourse.tile as tile
from concourse import bacc, mybir

# Make Exp and Ln resolve to the single combined ACT table set so the
# whole pre-MLP kernel needs one table load (the greedy per-function set
# picker would otherwise thrash natural_log <-> exp_and_others per LN).
import concourse.hw_specs as _hw_specs
import concourse.bacc as _bacc_mod
import concourse.bass_interp as _interp_mod

_orig_gat = _hw_specs.get_activation_tables


def _gat_combined(arch):
    _AF = mybir.ActivationFunctionType
    tables = {}
    for name, fns in _orig_gat(arch).items():
        fns = set(fns)
        if name != "natural_log_exp_and_others":
            fns.discard(_AF.Exp)
            fns.discard(_AF.Ln)
        tables[name] = fns
    return tables


_bacc_mod.get_activation_tables = _gat_combined
_interp_mod.get_activation_tables = _gat_combined

F32 = mybir.dt.float32
BF16 = mybir.dt.bfloat16
AF = mybir.ActivationFunctionType
OP = mybir.AluOpType

P = 128
D = 1024
H = 16
HD = 64
FF = 4096
TL = 512           # local (query) tokens per core
KC = D // P        # 8
NCORES = 8
EPS = 1e-5


def _swap_pairs(ap4):
    """View with the two elements of each innermost [step,2] pair swapped."""
    st = ap4.ap[-1][0]
    return bass.AP(
        tensor=ap4.tensor,
        offset=ap4.offset + st,
        ap=list(ap4.ap[:-1]) + [[-st, 2]],
    )


def build_program(nkc: int, apply_ln1: bool, apply_ln2: bool,
                  sim_compat: bool = False):
    nc = bacc.Bacc("TRN2", target_bir_lowering=False, debug=False)
    nk = nkc * P
    NQ = TL // P       # 4

    xg = nc.dram_tensor("xg", [nk, D], BF16, kind="ExternalInput").ap()
    xq = nc.dram_tensor("xq", [TL, D], F32, kind="ExternalInput").ap()
    mmz_d = nc.dram_tensor("mmz", [P, nkc, 2], F32, kind="ExternalInput").ap()
    trig_d = nc.dram_tensor("trig", [P, 2 * nkc + 8, HD], BF16,
                            kind="ExternalInput").ap()
    wq_d = nc.dram_tensor("wq", [P, KC, D], BF16, kind="ExternalInput").ap()
    wk_d = nc.dram_tensor("wk", [P, KC, D], BF16, kind="ExternalInput").ap()
    wv_d = nc.dram_tensor("wv", [P, KC, D], BF16, kind="ExternalInput").ap()
    wo_d = nc.dram_tensor("wo", [P, KC, D], BF16, kind="ExternalInput").ap()
    w1_d = nc.dram_tensor("w1", [P, KC, FF], BF16, kind="ExternalInput").ap()
    w2_d = nc.dram_tensor("w2", [P, FF // P, D], BF16, kind="ExternalInput").ap()
    ln_d = {nm: nc.dram_tensor(nm, [1, D], F32, kind="ExternalInput").ap()
            for nm in ("ln1w", "ln1b", "ln2w", "ln2b")}
    out_d = nc.dram_tensor("out", [TL, D], F32, kind="ExternalOutput").ap()
    zs_d = nc.dram_tensor("zstage", [H, TL], BF16, kind="Internal").ap()

    gelu_f = AF.Identity if sim_compat else AF.Gelu

    with tile.TileContext(nc) as tc:
        es0 = ExitStack()

        cons = es0.enter_context(tc.tile_pool(name="cons", bufs=1))
        work = es0.enter_context(tc.tile_pool(name="work", bufs=2))

        # ---- x loads first (they gate everything) ----
        pool_hT = tc.alloc_tile_pool(name="p_hT", bufs=1, side="right")
        pool_xg = tc.alloc_tile_pool(name="p_xg", bufs=1, side="right")
        pool_kT = tc.alloc_tile_pool(name="p_kT", bufs=1)
        pool_v1 = tc.alloc_tile_pool(name="p_v1", bufs=1)
        pool_qT = tc.alloc_tile_pool(name="p_qT", bufs=1)
        xts = []
        for ti in range(nkc):
            xt = pool_xg.tile([P, D], BF16, name=f"xt{ti}")
            nc.sync.dma_start(xt[:], xg[ti * P:(ti + 1) * P, :])
            xts.append(xt)
        pool_wB = tc.alloc_tile_pool(name="p_wB", bufs=1)
        wkh, wvh, wqh = [], [], []
        for nh in range(2):
            t = pool_wB.tile([P, KC, 512], BF16, name=f"wk{nh}")
            nc.sync.dma_start(t[:], wk_d[:, :, nh * 512:(nh + 1) * 512])
            wkh.append(t)
        # ---------------- constants ----------------
        eps_t = cons.tile([P, 1], F32, name="eps_t")
        nc.vector.memset(eps_t[:], EPS)

        # trig/mmz then xq on gpsimd (slow SWDGE, needed later)
        trig = cons.tile([P, 2 * nkc + 8, HD], BF16, name="trig_sb")
        nc.gpsimd.dma_start(trig[:], trig_d)
        cik = trig[:, 0:nkc, :]
        sik = trig[:, nkc:2 * nkc, :]
        ciq = trig[:, 2 * nkc:2 * nkc + 4, :]
        siq = trig[:, 2 * nkc + 4:2 * nkc + 8, :]
        mmz = cons.tile([P, nkc, 2], F32, name="mmz_sb")
        nc.gpsimd.dma_start(mmz[:], mmz_d)

        xqs = []
        for ti in range(NQ):
            xtq = work.tile([P, D], F32, tag="xtq", bufs=NQ, name=f"xtq{ti}")
            nc.gpsimd.dma_start(xtq[:], xq[ti * P:(ti + 1) * P, :])
            xqs.append(xtq)

        ln_bc = {}
        for nm, need in (("ln1w", apply_ln1), ("ln1b", apply_ln1),
                         ("ln2w", apply_ln2), ("ln2b", apply_ln2)):
            if need:
                t = cons.tile([P, D], F32, name=f"{nm}_bc")
                src = bass.AP(tensor=ln_d[nm].tensor, offset=ln_d[nm].offset,
                              ap=[[0, P], [1, D]])
                nc.gpsimd.dma_start(t[:], src)
                ln_bc[nm] = t

        def layernorm(src_ap, dst_ap, wkey, bkey, applied):
            """src [P, D] (any dtype) -> dst [P, D] bf16 normalized.

            rstd = exp(-0.5*ln(var+eps)): stays on the natural_log_exp ACT
            table set, so no table switches between LN / softmax-exp.
            """
            stats = work.tile([P, 2, 6], F32, tag="stats", bufs=4, name="st")
            nc.vector.bn_stats(stats[:, 0, :], src_ap[:, 0:512])
            nc.vector.bn_stats(stats[:, 1, :], src_ap[:, 512:1024])
            mv = work.tile([P, 2], F32, tag="mv", bufs=4, name="mv")
            nc.vector.bn_aggr(mv[:], stats[:])
            lnv = work.tile([P, 1], F32, tag="lnv", bufs=4, name="lnv")
            nc.scalar.activation(lnv[:], mv[:, 1:2], AF.Ln, bias=eps_t[:])
            rstd = work.tile([P, 1], F32, tag="rstd", bufs=4, name="rstd")
            nc.scalar.activation(rstd[:], lnv[:], AF.Exp, scale=-0.5)
            nc.vector.tensor_scalar(dst_ap, src_ap, mv[:, 0:1], rstd[:],
                                    OP.subtract, OP.mult)
            if applied:
                nc.vector.tensor_tensor(dst_ap, dst_ap, ln_bc[wkey][:], OP.mult)
                nc.vector.tensor_tensor(dst_ap, dst_ap, ln_bc[bkey][:], OP.add)

        def rope(krb, ci, si, ti, dst_ap, nh8):
            """RoPE a [P, nh8*64] bf16 tile -> dst bf16 sbuf."""
            w = nh8 * HD
            kb_h = krb.rearrange("p (h i) -> p h i", h=nh8)
            ci_b = ci[:, ti, None, :].to_broadcast((P, nh8, HD))
            p1 = work.tile([P, w], BF16, tag=f"p1_{nh8}", bufs=2, name="p1")
            nc.vector.tensor_tensor(p1.rearrange("p (h i) -> p h i", h=nh8),
                                    kb_h, ci_b, OP.mult)
            kb_sw = _swap_pairs(krb.rearrange("p (h i two) -> p h i two",
                                              h=nh8, two=2))
            si_b = (si[:, ti, None, :].to_broadcast((P, nh8, HD))
                    .rearrange("p h (i two) -> p h i two", two=2))
            p2 = work.tile([P, w], BF16, tag=f"p2_{nh8}", bufs=2, name="p2")
            nc.vector.tensor_tensor(
                p2.rearrange("p (h i two) -> p h i two", h=nh8, two=2),
                kb_sw, si_b, OP.mult)
            nc.vector.tensor_tensor(dst_ap, p1[:], p2[:], OP.add)

        def proj_group(dst_ps, lhs_base, wtile):
            for kc in range(KC):
                nc.tensor.matmul(dst_ps, lhsT=lhs_base[:, kc, :],
                                 rhs=wtile[:, kc, :],
                                 start=(kc == 0), stop=(kc == KC - 1))

        psB = tc.alloc_tile_pool(name="psB", bufs=1, space="PSUM")

        def ps512(nm):
            return psB.tile([P, 512], F32, tag="ps512", bufs=4, name=nm)

        # ========== stage A: LN1 of gathered key tokens -> hT ==========
        hT = pool_hT.tile([P, nkc, KC, P], BF16, name="hT")
        for ti in range(nkc):
            h = work.tile([P, D], BF16, tag="h", bufs=3, name=f"h{ti}")
            layernorm(xts[ti][:], h[:], "ln1w", "ln1b", apply_ln1)
            nc.sync.dma_start_transpose(hT[:, ti, :, :], h[:])
        pool_xg.release()

        # wv/wq halves on the scalar HWDGE queue, after stage A's LN work
        for nm, lst, srcd in (("wv", wvh, wv_d), ("wq", wqh, wq_d)):
            for nh in range(2):
                t = pool_wB.tile([P, KC, 512], BF16, name=f"{nm}{nh}")
                nc.scalar.dma_start(t[:], srcd[:, :, nh * 512:(nh + 1) * 512])
                lst.append(t)

        # ========== stage B: k, v (gathered keys), q (local) ==========
        kT = pool_kT.tile([P, nkc, KC, P], BF16, name="kT")
        for ti in range(nkc):
            krb = work.tile([P, D], BF16, tag="krb", bufs=2, name=f"krb{ti}")
            for nh in range(2):
                ps = ps512(f"kps{nh}_{ti}")
                proj_group(ps, hT[:, ti, :, :], wkh[nh])
                nc.scalar.copy(krb[:, nh * 512:(nh + 1) * 512], ps[:])
            kr = work.tile([P, D], BF16, tag="kr", bufs=2, name=f"kr{ti}")
            rope(krb[:], cik, sik, ti, kr[:], 16)
            nc.sync.dma_start_transpose(kT[:, ti, :, :], kr[:])

        # q path: nh-outer so scores for head-pairs 0-3 can start while the
        # nh=1 projections still run
        qT = pool_qT.tile([P, NQ, KC, P], BF16, name="qT")
        hqTs = []
        for ti in range(NQ):
            hq = work.tile([P, D], BF16, tag="h", bufs=3, name=f"hq{ti}")
            layernorm(xqs[ti][:], hq[:], "ln1w", "ln1b", apply_ln1)
            hqT = work.tile([P, KC, P], BF16, tag="hqT", bufs=NQ,
                            name=f"hqT{ti}")
            nc.sync.dma_start_transpose(hqT[:], hq[:])
            hqTs.append(hqT)
        for nh in range(2):
            for ti in range(NQ):
                ps = ps512(f"qps{nh}_{ti}")
                proj_group(ps, hqTs[ti], wqh[nh])
                qrb = work.tile([P, 512], BF16, tag="qrb", bufs=2,
                                name=f"qrb{nh}_{ti}")
                nc.scalar.copy(qrb[:], ps[:])
                qr = work.tile([P, 512], BF16, tag="qr", bufs=3,
                               name=f"qr{nh}_{ti}")
                rope(qrb[:], ciq, siq, ti, qr[:], 8)
                nc.sync.dma_start_transpose(
                    qT[:, ti, nh * 4:(nh + 1) * 4, :], qr[:])

        v1 = pool_v1.tile([P, nkc, H, 66], BF16, name="v1")
        # v1 mask columns: col 64 = keep flag (1/0), col 65 = 0 (pad)
        for ti in range(nkc):
            nc.vector.tensor_copy(v1[:, ti, :, 64:66],
                                  mmz[:, ti, None, :].to_broadcast((P, H, 2)))
        for nh in range(2):
            for ti in range(nkc):
                ps = ps512(f"vps{nh}_{ti}")
                proj_group(ps, hT[:, ti, :, :], wvh[nh])
                # eviction with per-token keep-scale on the ACT engine
                nc.scalar.activation(
                    v1[:, ti, nh * 8:(nh + 1) * 8, 0:64],
                    ps.rearrange("p (h d) -> p h d", h=8),
                    AF.Copy, scale=mmz[:, ti, 0:1])

        pool_wB.release()
        pool_hT.release()
        psB.release()

        # ========== stage C: attention per head-pair ==========
        pool_wD = tc.alloc_tile_pool(name="p_wD", bufs=1, side="right")
        woh = []
        for nh in range(2):
            t = pool_wD.tile([P, KC, 512], BF16, name=f"wo{nh}")
            nc.gpsimd.dma_start(t[:], wo_d[:, :, nh * 512:(nh + 1) * 512])
            woh.append(t)

        psC = tc.alloc_tile_pool(name="psC", bufs=1, space="PSUM")
        pool_at = tc.alloc_tile_pool(name="p_at", bufs=1, side="right")
        pool_pT = tc.alloc_tile_pool(name="p_pT", bufs=2, side="right")
        pool_z = tc.alloc_tile_pool(name="p_z", bufs=1, side="right")
        pool_zbc = tc.alloc_tile_pool(name="p_zbc", bufs=1, side="right")
        attnT = pool_at.tile([P, KC, TL], BF16, name="attnT")
        ztabs = {}

        def extract_z(pj, pvv):
            # Z rows -> ztab rows {0,32,64,96} (DVE)
            for eo in range(2):
                hh = 2 * pj + eo
                hg, hi = divmod(hh, 4)
                if hi == 0:
                    ztabs[hg] = pool_z.tile([P, TL], F32, tag="ztab", bufs=2,
                                            name=f"ztab{hg}")
                    nc.vector.memset(ztabs[hg][:], 1.0)
                nc.vector.tensor_copy(ztabs[hg][32 * hi:32 * hi + 1, :],
                                      pvv[64:65, eo * 512:(eo + 1) * 512])

        def recip_stage(hg):
            # batched reciprocal for 4 heads (rows 0/32/64/96); cast to
            # bf16; stage through DRAM
            ztab = ztabs.pop(hg)
            nc.vector.reciprocal(ztab[:], ztab[:])
            zcast = pool_z.tile([P, TL], BF16, tag="zcast", bufs=2,
                                name=f"zcast{hg}")
            nc.vector.tensor_copy(zcast[:], ztab[:])
            for hi in range(4):
                hh = hg * 4 + hi
                dst = bass.AP(tensor=zs_d.tensor,
                              offset=zs_d.offset + hh * TL,
                              ap=[[TL, 1], [1, TL]])
                nc.sync.dma_start(dst, zcast[32 * hi:32 * hi + 1, :])

        def evict_attn(pj, pvv):
            # unnormalized attnT evict (DVE, bf16 cast)
            for eo in range(2):
                pb = 64 * eo
                nc.vector.tensor_copy(attnT[pb:pb + 64, pj, :],
                                      pvv[0:64, eo * 512:(eo + 1) * 512])

        def emit_normalize(hg):
            # broadcast 1/Z for 4 heads in one DMA; in-place bf16 normalize
            zbc = pool_zbc.tile([P, 4, TL], BF16, tag="zbc", bufs=3,
                                name=f"zbc{hg}")
            src = bass.AP(tensor=zs_d.tensor, offset=zs_d.offset + hg * 4 * TL,
                          ap=[[0, P], [TL, 4], [1, TL]])
            nc.sync.dma_start(zbc[:], src)
            for hi in range(4):
                hh = hg * 4 + hi
                pj, eo = divmod(hh, 2)
                pb = 64 * eo
                nc.vector.tensor_tensor(attnT[pb:pb + 64, pj, :],
                                        attnT[pb:pb + 64, pj, :],
                                        zbc[pb:pb + 64, hi, :], OP.mult)

        prev = None
        for j in range(H // 2 + 1):
            last = j == H // 2
            if not last:
                pt = pool_pT.tile([P, nkc, 2, 512], BF16, tag="pT", bufs=3,
                                  name=f"pT{j}")
            for skc in range(nkc):
                if not last:
                    pss = psC.tile([P, 1024], F32, tag="sc", bufs=2,
                                   name=f"scps{j}_{skc}")
                    for eo in range(2):
                        pb = 64 * eo
                        nc.tensor.matmul(
                            pss[:, eo * 512:(eo + 1) * 512],
                            lhsT=kT[pb:pb + 64, skc, j, :],
                            rhs=qT[pb:pb + 64, :, j, :],
                            start=True, stop=True)
                    nc.scalar.activation(
                        pt[:, skc, :, :].rearrange("p a b -> p (a b)"),
                        pss[:], AF.Exp, scale=0.125)
                if prev is not None:
                    pj, ppt, pvv = prev
                    for eo in range(2):
                        nc.tensor.matmul(
                            pvv[0:66, eo * 512:(eo + 1) * 512],
                            lhsT=v1[:, skc, 2 * pj + eo, :],
                            rhs=ppt[:, skc, eo, :],
                            start=(skc == 0), stop=(skc == nkc - 1))
            if prev is not None:
                pj = prev[0]
                extract_z(pj, prev[2])
                if pj % 2 == 1:
                    recip_stage(pj // 2)
                evict_attn(pj, prev[2])
                if pj % 2 == 1:
                    emit_normalize(pj // 2)
            if not last:
                prev = (j, pt, psC.tile([P, 1024], F32, tag="pv", bufs=2,
                                        name=f"pv{j}"))

        pool_zbc.release()
        pool_z.release()
        pool_pT.release()
        pool_qT.release()
        pool_v1.release()
        pool_kT.release()
        psC.release()

        # ========== stage D: wo + residual -> xres; LN2 -> h2T ==========
        psD = tc.alloc_tile_pool(name="psD", bufs=1, space="PSUM")
        pool_res = tc.alloc_tile_pool(name="p_res", bufs=1)
        xres = pool_res.tile([P, NQ, D], F32, name="xres")
        h2T = pool_res.tile([P, NQ, KC, P], BF16, name="h2T")
        pool_w1 = tc.alloc_tile_pool(name="p_w1", bufs=2)

        def psDt(nm):
            return psD.tile([P, 512], F32, tag="wops", bufs=2, name=nm)

        # wo split: heads 0-11 (kc 0-5) are normalized well before heads
        # 12-15 (kc 6-7); run the early part as soon as psum frees, then
        # only the 2-chunk remainder waits on the last softmax group
        for tc4 in range(NQ):
            for nh in range(2):
                ps = psDt(f"woA{tc4}_{nh}")
                for kc in range(6):
                    nc.tensor.matmul(
                        ps[:], lhsT=attnT[:, kc, tc4 * P:(tc4 + 1) * P],
                        rhs=woh[nh][:, kc, :],
                        start=(kc == 0), stop=(kc == 5))
                nc.vector.tensor_add(xres[:, tc4, nh * 512:(nh + 1) * 512],
                                     ps[:],
                                     xqs[tc4][:, nh * 512:(nh + 1) * 512])
        for tc4 in range(NQ):
            for nh in range(2):
                ps = psDt(f"woB{tc4}_{nh}")
                for kc in range(6, KC):
                    nc.tensor.matmul(
                        ps[:], lhsT=attnT[:, kc, tc4 * P:(tc4 + 1) * P],
                        rhs=woh[nh][:, kc, :],
                        start=(kc == 6), stop=(kc == KC - 1))
                nc.vector.tensor_add(xres[:, tc4, nh * 512:(nh + 1) * 512],
                                     ps[:],
                                     xres[:, tc4, nh * 512:(nh + 1) * 512])
            h2 = work.tile([P, D], BF16, tag="h", bufs=3, name=f"h2{tc4}")
            layernorm(xres[:, tc4, :], h2[:], "ln2w", "ln2b", apply_ln2)
            nc.sync.dma_start_transpose(h2T[:, tc4, :, :], h2[:])

        pool_at.release()
        pool_wD.release()
        psD.release()
        psE = tc.alloc_tile_pool(name="psE", bufs=1, space="PSUM")

        # ========== stage E: MLP ==========
        pool_g1 = tc.alloc_tile_pool(name="p_g1", bufs=1, side="right")
        pool_w2 = tc.alloc_tile_pool(name="p_w2", bufs=2, side="right")
        g1 = pool_g1.tile([P, FF // P, TL], BF16, name="g1")

        for fg in range(FF // 1024):
            w1c = pool_w1.tile([P, KC, 1024], BF16, tag="w1c", name=f"w1c{fg}")
            nc.scalar.dma_start(w1c[:], w1_d[:, :, fg * 1024:(fg + 1) * 1024])
            for j in range(8):
                ps = psE.tile([P, 512], F32, tag="m1ps", bufs=4,
                              name=f"m1ps{fg}_{j}")
                # split over tc-halves so the first MMs start before the
                # last h2T tile lands
                for c2 in range(2):
                    for kc in range(KC):
                        nc.tensor.matmul(
                            ps[:, c2 * 256:(c2 + 1) * 256],
                            lhsT=w1c[:, kc, j * P:(j + 1) * P],
                            rhs=h2T[:, 2 * c2:2 * c2 + 2, kc, :],
                            start=(kc == 0), stop=(kc == KC - 1))
                nc.scalar.activation(g1[:, fg * 8 + j, :], ps[:], gelu_f)

        ots = []
        for tc4 in range(NQ):
            ots.append(work.tile([P, D], F32, tag="osb", bufs=NQ,
                                 name=f"ot{tc4}"))
        for nh in range(2):
            psos = [psE.tile([P, 512], F32, tag="m2ps", bufs=4,
                             name=f"m2ps{nh}_{tc4}") for tc4 in range(NQ)]
            for kg in range(4):
                w2c = pool_w2.tile([P, 8, 512], BF16, tag="w2c",
                                   name=f"w2c{nh}_{kg}")
                nc.scalar.dma_start(
                    w2c[:], w2_d[:, kg * 8:(kg + 1) * 8,
                                 nh * 512:(nh + 1) * 512])
                for tc4 in range(NQ):
                    for kc in range(8):
                        nc.tensor.matmul(
                            psos[tc4],
                            lhsT=g1[:, kg * 8 + kc, tc4 * P:(tc4 + 1) * P],
                            rhs=w2c[:, kc, :],
                            start=(kg == 0 and kc == 0),
                            stop=(kg == 3 and kc == 7))
            for tc4 in range(NQ):
                nc.vector.tensor_add(
                    ots[tc4][:, nh * 512:(nh + 1) * 512], psos[tc4][:],
                    xres[:, tc4, nh * 512:(nh + 1) * 512])
        for tc4 in range(NQ):
            nc.sync.dma_start(out_d[tc4 * P:(tc4 + 1) * P, :], ots[tc4][:])

        pool_w2.release()
        pool_g1.release()
        pool_w1.release()
        pool_res.release()
        psE.release()
        es0.close()

    nc.compile()
    return nc


# ---------------------------------------------------------------------------
# Host side
# ---------------------------------------------------------------------------

_PROGRAM_CACHE = {}


def _get_program(nkc, apply_ln1, apply_ln2, sim_compat=False):
    key = (nkc, apply_ln1, apply_ln2, sim_compat)
    if key not in _PROGRAM_CACHE:
        _PROGRAM_CACHE[key] = build_program(*key)
    return _PROGRAM_CACHE[key]


def _prep_inputs(x, mask, freqs_cos, freqs_sin, wq, wk, wv, wo, w1, w2,
                 ln1_w, ln1_b, ln2_w, ln2_b):
    """Build the 8 per-core input dicts.  Returns (in_maps, nkc)."""
    f32 = np.float32
    bf16 = ml_dtypes.bfloat16
    x = np.asarray(x, f32)
    mask = np.asarray(mask)
    cos = np.asarray(freqs_cos, f32)
    sin = np.asarray(freqs_sin, f32)
    S = x.shape[1]

    ci = np.empty((S, HD), f32)
    ci[:, 0::2] = cos
    ci[:, 1::2] = cos
    si = np.empty((S, HD), f32)
    si[:, 0::2] = -sin
    si[:, 1::2] = sin

    # gather unmasked key tokens per batch, pad to a common multiple of 128
    idxs, keeps = [], []
    nkc = 1
    for b in range(x.shape[0]):
        idx = np.nonzero(~mask[b])[0]
        nkc = max(nkc, -(-max(len(idx), 1) // P))
        idxs.append(idx)
    nk = nkc * P
    for b in range(x.shape[0]):
        idx = idxs[b]
        n = len(idx)
        pad = np.zeros(nk - n, dtype=np.int64)
        idxs[b] = np.concatenate([idx, pad])
        keeps.append(np.concatenate([np.ones(n, f32), np.zeros(nk - n, f32)]))

    def wlayout(w, kc):
        w = np.asarray(w, f32)
        return np.ascontiguousarray(
            w.reshape(kc, P, w.shape[1]).transpose(1, 0, 2)).astype(bf16)

    shared = {
        "wq": wlayout(wq, KC), "wk": wlayout(wk, KC), "wv": wlayout(wv, KC),
        "wo": wlayout(wo, KC), "w1": wlayout(w1, KC),
        "w2": wlayout(w2, FF // P),
        "ln1w": np.asarray(ln1_w, f32).reshape(1, D),
        "ln1b": np.asarray(ln1_b, f32).reshape(1, D),
        "ln2w": np.asarray(ln2_w, f32).reshape(1, D),
        "ln2b": np.asarray(ln2_b, f32).reshape(1, D),
    }

    def tposed(a, nchunks):  # [n*P, w] -> [P, n, w]
        return np.ascontiguousarray(
            a.reshape(nchunks, P, a.shape[1]).transpose(1, 0, 2))

    in_maps = []
    for c in range(NCORES):
        b, half = divmod(c, 2)
        idx, keep = idxs[b], keeps[b]
        m = dict(shared)
        m["xg"] = np.ascontiguousarray(x[b][idx]).astype(bf16)
        m["xq"] = np.ascontiguousarray(x[b, half * TL:(half + 1) * TL])
        mmp = keep.reshape(nkc, P).T  # [P, nkc]
        m["mmz"] = np.ascontiguousarray(
            np.stack([mmp, np.zeros_like(mmp)], axis=-1))
        trig = np.concatenate([
            tposed(ci[idx], nkc), tposed(si[idx], nkc),
            tposed(ci[half * TL:(half + 1) * TL], 4),
            tposed(si[half * TL:(half + 1) * TL], 4)], axis=1)
        m["trig"] = trig.astype(bf16)
        in_maps.append(m)
    return in_maps, nkc


def kernel(x, mask, freqs_cos, freqs_sin, wq, wk, wv, wo, w1, w2,
           ln1_w, ln1_b, ln2_w, ln2_b, _trace=False, _sim=False):
    from concourse.bass_utils import run_bass_kernel_spmd

    apply_ln1 = not (np.all(np.asarray(ln1_w) == 1.0)
                     and np.all(np.asarray(ln1_b) == 0.0))
    apply_ln2 = not (np.all(np.asarray(ln2_w) == 1.0)
                     and np.all(np.asarray(ln2_b) == 0.0))
    in_maps, nkc = _prep_inputs(x, mask, freqs_cos, freqs_sin, wq, wk, wv, wo,
                                w1, w2, ln1_w, ln1_b, ln2_w, ln2_b)
    nc = _get_program(nkc, apply_ln1, apply_ln2, sim_compat=_sim)

    if _sim:
        from concourse.bass_interp import CoreSim
        sim = CoreSim(nc, trace=False)
        for k, v in in_maps[0].items():
            sim.tensor(k)[:] = v
        sim.simulate(check_with_hw=False)
        B, S = x.shape[0], x.shape[1]
        full = np.empty((B, S, D), np.float32)
        full[0, :TL] = np.array(sim.tensor("out"))
        return full

    res = run_bass_kernel_spmd(nc, in_maps, core_ids=list(range(NCORES)),
                               trace=_trace)
    B, S = x.shape[0], x.shape[1]
    full = np.empty((B, S, D), np.float32)
    for c in range(NCORES):
        b, half = divmod(c, 2)
        full[b, half * TL:(half + 1) * TL] = res.results[c]["out"]
    if _trace:
        return full, res
    return full
